# revision 7
# baseline (speedup 1.0000x reference)
"""Multi-head attention (multiquery K/V) Bass kernel for 8 trn2 NeuronCores.

Sharding: 8 cores = 2 batches x 4 query-row quarters. Each core computes the
full multiquery K/V projections for its batch (cheap, dk=64) and runs
attention + output projection for its 512 query rows over all 16 heads.
Output is a pure concatenation across cores -- no collectives.

Design (v4):
- Steady state is jointly PE/ACT bound: per t-block the PE runs 2 score +
  2 attn@V matmuls (512 cols each) plus amortized projection hooks, the
  Scalar engine one exp over [128, 2, 512].  Everything streams bf16 (the
  fp32r paths of v3 bought nothing on HW and doubled LDWEIGHTS).
- Scores keep the full-128 contraction, but the zero padding lives in the
  STATIONARY operand now: K2Tz[:,0]=[K;0], K2Tz[:,1]=[0;K], and qz holds the
  head pair stacked [Qe;Qo] with no zero halves (no big memzero, single
  evacuation copy per q-block).
- attn@V lags exp by 2 t-blocks everywhere (ex bufs=4): the previous pass's
  last two attn@V land in tb0/tb1 of the next pass, so no 6-matmul burst at
  pass boundaries.  The accumulator evacuates (split per j) at tb1; the new
  pass's attn@V starts at tb2.
- Normalize: reciprocal_approx_fast on psum row 0 -> rec1 [1,2,512]; a
  K=1-contraction ones-stationary matmul broadcasts 1/den across partitions
  (no 65-row zero region needed); DVE multiplies write the pair-stacked oT.
- Startup: warm-up matmuls on a junk tile un-throttle the PE HAM and an
  early dummy activation preloads the exp table set, both during the DMA
  wait; input DMAs issue from 4 engines in first-use order with xc0 split
  in halves so the first projection starts ~5us earlier.
- Tail: deferred attn@V right after the last exp, j-split reciprocal, bc
  evacuation on the (now idle) Scalar engine, and per-block y matmul ->
  add -> DMA pipeline.
"""

import sys

import numpy as np

if "/opt/trn_rl_repo" not in sys.path:
    sys.path.insert(0, "/opt/trn_rl_repo")

B, S, D = 2, 2048, 1024
H, DK = 16, 64
H2 = H // 2  # head pairs
P = 128
NCORES, GPB = 8, 4
SPB = S // GPB  # 512 query rows per core
KC = D // P  # 8 contraction subtiles over d_model
NT = S // P  # 16 key/t blocks
NSB = SPB // P  # 4 s blocks


def build_bass(scale: float, debug: bool = False):
    import concourse.bacc as bacc
    import concourse.mybir as mybir
    import concourse.tile as tile
    from concourse.bass import ts
    from concourse.dve_ops import (
        RECIP_APPROX_FAST_CONSTS,
        RECIPROCAL_APPROX_FAST,
    )

    fp32 = mybir.dt.float32
    mdt = mybir.dt.float32r
    Act = mybir.ActivationFunctionType

    bf16 = mybir.dt.bfloat16
    nc = bacc.Bacc(None, target_bir_lowering=False)
    xT = nc.dram_tensor("xT", [D, S], bf16, kind="ExternalInput")
    cst = nc.dram_tensor("cst", [P, 256], mdt, kind="ExternalInput")
    cstb = nc.dram_tensor("cstb", [P, P], bf16, kind="ExternalInput")
    wqT = nc.dram_tensor("wqT", [D, D], bf16, kind="ExternalInput")
    wkT = nc.dram_tensor("wkT", [D, DK], bf16, kind="ExternalInput")
    wvT = nc.dram_tensor("wvT", [D, DK + 1], bf16, kind="ExternalInput")
    wo2 = nc.dram_tensor("wo2", [P, H2, D], bf16, kind="ExternalInput")
    y = nc.dram_tensor("y", [SPB, D], bf16, kind="ExternalOutput")

    xT3 = xT.rearrange("(po pi) s -> pi po s", pi=P)
    wq3 = wqT.rearrange("(po pi) d -> pi po d", pi=P)
    wk3 = wkT.rearrange("(po pi) d -> pi po d", pi=P)
    wv3 = wvT.rearrange("(po pi) d -> pi po d", pi=P)

    with tile.TileContext(nc) as tc:
        with (
            tc.tile_pool(name="sb", bufs=1) as sb,
            tc.tile_pool(name="ps", bufs=1, space="PSUM") as ps,
        ):
            # ---- persistent SBUF ----
            cst_sb = sb.tile([P, 256], mdt, name="cst")
            cstb_sb = sb.tile([P, P], bf16, name="cstb")
            K2Tz = sb.tile([P, 2, S], bf16, name="K2Tz")
            # Vp stationary: col 0 = denominator ones column, cols 32:96 = V'
            Vp = sb.tile([P, NT, 96], bf16, name="Vp")
            qz = sb.tile([P, KC, SPB], bf16, name="qz")
            rec1 = sb.tile([1, 2, SPB], mdt, name="rec1")
            acc_sb = sb.tile([96, 2, SPB], fp32, name="acc_sb")
            y_sb = sb.tile([P, NSB, D], fp32, name="y_sb")
            wk_sb = sb.tile([P, KC, DK], bf16, name="wk")
            wv_sb = sb.tile([P, KC, DK + 1], bf16, name="wv")
            wq_sb = sb.tile([P, KC, D], bf16, name="wq")
            wo2_sb = sb.tile([P, H2, D], bf16, name="wo2")
            xc0 = sb.tile([P, KC, SPB], bf16, name="xc0")
            junk = sb.tile([P, SPB], bf16, name="junk")
            dead = sb.tile([P, 8], fp32, name="dead")

            ident = cstb_sb[0:DK, 0:DK]
            ones1 = cst_sb[0:1, DK:DK + 96]  # [1, 96] of ones

            def aux(shape, dtype=fp32):
                return ps.tile(shape, dtype, name="aux", tag="aux", bufs=2)

            # ---- warm-up: junk matmuls un-throttle the PE HAM while DMAs
            # stream; a 1-col exp preloads the activation table set ----
            nc.vector.memset(junk[:], 0)
            nc.scalar.activation(dead[:, 0:1], junk[:, 0:1], Act.Exp,
                                 scale=1.0)
            for _ in range(10):
                wps = aux([P, SPB])
                nc.tensor.matmul(wps[:], junk[:, 0:P], junk[:],
                                 start=True, stop=True)

            # ---- input DMAs, spread over 4 engines in first-use order ----
            # gpsimd: the x stream (its queue carries the bulk)
            nc.gpsimd.dma_start(wk_sb[:], wk3[:])
            nc.gpsimd.dma_start(xc0[:, 0:4, :], xT3[:, 0:4, 0:SPB])
            nc.gpsimd.dma_start(xc0[:, 4:KC, :], xT3[:, 4:KC, 0:SPB])
            xcs = {0: xc0}
            xc1 = sb.tile([P, KC, SPB], bf16, name="xc", tag="xc", bufs=2)
            nc.gpsimd.dma_start(xc1[:], xT3[:, :, ts(1, SPB)])
            xcs[1] = xc1
            xc2 = sb.tile([P, KC, SPB], bf16, name="xc", tag="xc", bufs=2)
            nc.gpsimd.dma_start(xc2[:], xT3[:, :, ts(2, SPB)])
            xcs[2] = xc2
            xc3 = sb.tile([P, KC, SPB], bf16, name="xc", tag="xc", bufs=2)
            nc.gpsimd.dma_start(xc3[:], xT3[:, :, ts(3, SPB)])
            xcs[3] = xc3
            # sync: q/v weights for the pre-pass, constants, late q weights
            nc.sync.dma_start(wq_sb[:, :, ts(0, P)], wq3[:, :, ts(0, P)])
            nc.sync.dma_start(wv_sb[:], wv3[:])
            nc.sync.dma_start(wq_sb[:, :, ts(1, P)], wq3[:, :, ts(1, P)])
            nc.sync.dma_start(cstb_sb[:], cstb[:])
            nc.sync.dma_start(Vp[:, :, 0], cstb[:, DK:DK + NT])  # ones col
            nc.sync.dma_start(cst_sb[:], cst[:])
            for m in range(2, KC):
                nc.sync.dma_start(wq_sb[:, :, ts(m, P)], wq3[:, :, ts(m, P)])
            # scalar: w_out (first used at pass0 tb7)
            for hp in range(H2):
                nc.scalar.dma_start(wo2_sb[:, hp, :], wo2[:, hp, :])

            # ---- one-time zero regions (off the DVE/ACT critical path) ----
            # K2Tz stationary zero halves.  Vp cols 1:32 stay uninitialized:
            # their attn@V psum rows 1:32 are never read.
            nc.vector.memset(K2Tz[DK:P, 0, :], 0)
            nc.vector.memset(K2Tz[0:DK, 1, :], 0)

            # projection emitters, split into <=1us pieces so they slot into
            # per-tb PE slack without stalling the exp cadence; piece "a"
            # starts the psum accumulation, "b" finishes it and copies out
            pstate = {}

            def k2_a(c):
                kps = aux([DK, SPB])
                pstate[("k2", c)] = kps
                for k in range(4):
                    nc.tensor.matmul(
                        kps[:], wk_sb[:, k, :], xcs[c][:, k, :],
                        start=(k == 0), stop=False,
                    )

            def k2_b(c):
                kps = pstate.pop(("k2", c))
                for k in range(4, KC):
                    nc.tensor.matmul(
                        kps[:], wk_sb[:, k, :], xcs[c][:, k, :],
                        start=False, stop=(k == KC - 1),
                    )
                nc.vector.tensor_copy(K2Tz[0:DK, 0, ts(c, 512)], kps[:])
                nc.vector.tensor_copy(K2Tz[DK:P, 1, ts(c, 512)], kps[:])

            def v_a(c):
                vps = aux([DK + 1, SPB])
                pstate[("v", c)] = vps
                for k in range(4):
                    nc.tensor.matmul(
                        vps[:], wv_sb[:, k, :], xcs[c][:, k, :],
                        start=(k == 0), stop=False,
                    )

            def v_b(c):
                vps = pstate.pop(("v", c))
                for k in range(4, KC):
                    nc.tensor.matmul(
                        vps[:], wv_sb[:, k, :], xcs[c][:, k, :],
                        start=False, stop=(k == KC - 1),
                    )
                vsb = sb.tile([DK, SPB], bf16, name="vsb", tag="vsb", bufs=1)
                pstate[("vsb", c)] = vsb
                nc.vector.tensor_copy(vsb[:], vps[0:DK, :])

            def v_tr(c):
                # PE-transpose V.T -> V' [t, dv] into Vp cols 32:96
                vsb = pstate.pop(("vsb", c))
                for pair in range(2):
                    trs = [aux([P, DK], bf16), aux([P, DK], bf16)]
                    for q in range(2):
                        nc.tensor.transpose(
                            trs[q][:], vsb[:, ts(2 * pair + q, P)], ident
                        )
                    for q in range(2):
                        nc.vector.tensor_copy(
                            Vp[:, 4 * c + 2 * pair + q, 32:96], trs[q][:]
                        )

            def q_a(m):
                qps = aux([P, SPB])
                pstate[("q", m)] = qps
                for k in range(4):
                    nc.tensor.matmul(
                        qps[:], wq_sb[:, k, ts(m, P)], xc0[:, k, :],
                        start=(k == 0), stop=False,
                    )

            def q_b(m):
                qps = pstate.pop(("q", m))
                for k in range(4, KC):
                    nc.tensor.matmul(
                        qps[:], wq_sb[:, k, ts(m, P)], xc0[:, k, :],
                        start=False, stop=(k == KC - 1),
                    )
                nc.vector.tensor_copy(qz[:, m, :], qps[:])

            # ---- pre-pass: K2/V for c0 and Q for m0, m1 ----
            k2_a(0)
            k2_b(0)
            q_a(0)
            q_b(0)
            q_a(1)
            q_b(1)
            v_a(0)
            v_b(0)
            v_tr(0)

            # ---- attention passes, one head pair each ----
            def emit_av(acc, tb, ex):
                first, last = (tb == 0), (tb == NT - 1)
                for j in range(2):
                    nc.tensor.matmul(
                        acc[:, j, :], Vp[:, tb, :], ex[:, j, :],
                        start=first, stop=last,
                    )

            def evac(prev_acc, j):
                nc.vector.tensor_copy(acc_sb[:, j, :], prev_acc[:, j, :])

            def norm_a(j=None):
                # reciprocal of the denominator row (the offset-dropping
                # custom op needs base partition 0 -- acc_sb row 0 is it)
                c = RECIP_APPROX_FAST_CONSTS
                if j is None:
                    o, i = rec1[0:1, :, :], acc_sb[0:1, :, :]
                else:
                    o, i = rec1[0:1, j, :], acc_sb[0:1, j, :]
                nc.vector._custom_dve(
                    RECIPROCAL_APPROX_FAST, out=o, in0=i,
                    s0=c["s0"], s1=c["s1"], imm2=c["imm2"],
                )

            def norm_b_j(j, oT, tail=False):
                # broadcast 1/den across partitions via a K=1-contraction
                # ones-stationary matmul, then DVE multiplies write the
                # pair-stacked normalized oT.  32-row pieces: spans starting
                # at partition 32/96 may cover at most 32 partitions, and
                # SBUF+SBUF inputs must share a base partition.
                bc = aux([96, SPB])
                nc.tensor.matmul(
                    bc[:], ones1, rec1[0:1, j, :], start=True, stop=True
                )
                bcs = sb.tile([96, SPB], fp32, name="bcs", tag="vsb", bufs=1)
                # psum reads starting at partition 32 may cover at most 32
                # partitions, so evacuate from partition 0
                if tail:
                    nc.scalar.copy(bcs[:], bc[:])
                else:
                    nc.vector.tensor_copy(bcs[:], bc[:])
                for hf in range(2):
                    nc.vector.tensor_mul(
                        oT[j * DK + hf * 32:j * DK + hf * 32 + 32, :],
                        acc_sb[32 + hf * 32:64 + hf * 32, j, :],
                        bcs[32 + hf * 32:64 + hf * 32, :],
                    )

            def norm_b(tail=False):
                oT = sb.tile([P, SPB], bf16, name="oT", tag="oT", bufs=2)
                for j in range(2):
                    norm_b_j(j, oT, tail=tail)
                return oT

            def emit_y(prev_hp, oT, sb4):
                yps = [aux([P, 512]), aux([P, 512])]
                for df in range(2):
                    nc.tensor.matmul(
                        yps[df][:], oT[:, ts(sb4, P)],
                        wo2_sb[:, prev_hp, ts(df, 512)],
                        start=True, stop=True,
                    )
                for df in range(2):
                    if prev_hp == 0:
                        nc.vector.tensor_copy(
                            y_sb[:, sb4, ts(df, 512)], yps[df][:]
                        )
                    else:
                        nc.vector.tensor_add(
                            y_sb[:, sb4, ts(df, 512)], yps[df][:],
                            y_sb[:, sb4, ts(df, 512)],
                        )

            def q_p(m, i):
                if i == 0:
                    pstate[("q", m)] = aux([P, SPB])
                qps = pstate[("q", m)]
                for k in (2 * i, 2 * i + 1):
                    nc.tensor.matmul(
                        qps[:], wq_sb[:, k, ts(m, P)], xc0[:, k, :],
                        start=(k == 0), stop=(k == KC - 1),
                    )

            def q_fin(m):
                qps = pstate.pop(("q", m))
                nc.vector.tensor_copy(qz[:, m, :], qps[:])

            # hook schedule: {pass: {tb: [closures]}}.  Deadlines: K2Tz
            # block c by sc(4c); Vp block c by av(4c) (tb 4c+2, lag 2);
            # qz m by the next pass's sc(0).  v_tr(c) must be emitted at
            # least one tb before av(4c) so its PE transposes precede the
            # dependent attn@V in the engine queue.
            hooks = {
                0: {0: [lambda: k2_a(1)], 1: [lambda: k2_b(1)],
                    2: [lambda: v_a(1)], 3: [lambda: v_b(1)],
                    4: [lambda: v_tr(1)],
                    5: [lambda: k2_a(2)], 6: [lambda: k2_b(2)],
                    7: [lambda: v_a(2)], 8: [lambda: v_b(2)],
                    9: [lambda: v_tr(2), lambda: k2_a(3)],
                    10: [lambda: k2_b(3)],
                    11: [lambda: v_a(3)], 12: [lambda: v_b(3)],
                    13: [lambda: v_tr(3)]},
            }
            for p in range(1, 7):
                for i in range(4):
                    hooks.setdefault(p, {}).setdefault(10 + i, []).append(
                        lambda m=p + 1, i=i: q_p(m, i)
                    )
                hooks.setdefault(p, {}).setdefault(14, []).append(
                    lambda m=p + 1: q_fin(m)
                )

            prev = None  # (hp, acc, tail exs {14, 15})
            for hp in range(H2):
                acc = ps.tile([96, 2, SPB], fp32, name="acc", tag="acc",
                              bufs=1)
                exs = {}
                oT_prev = None
                for tb in range(NT):
                    sc = ps.tile([P, 2, SPB], fp32, name=f"sc{tb % 2}",
                                 tag=f"sc{tb % 2}", bufs=1)
                    for j in range(2):
                        nc.tensor.matmul(
                            sc[:, j, :], K2Tz[:, j, ts(tb, P)], qz[:, hp, :],
                            start=True, stop=True,
                        )
                    # attn@V lags exp by 2; the previous pass's last two
                    # attn@V land in tb0/tb1, its accumulator evacuates at
                    # tb1 (split per j) and this pass's attn@V starts at tb2.
                    if tb == 0 and prev is not None:
                        emit_av(prev[1], 14, prev[2].pop(14))
                    if tb == 1 and prev is not None:
                        emit_av(prev[1], 15, prev[2].pop(15))
                    if tb >= 2:
                        emit_av(acc, tb - 2, exs.pop(tb - 2))
                    ex = sb.tile([P, 2, SPB], bf16, name="ex", tag="ex",
                                 bufs=4)
                    nc.scalar.activation(ex[:], sc[:], Act.Exp, scale=scale)
                    exs[tb] = ex
                    if tb == 1 and prev is not None:
                        evac(prev[1], 0)
                        evac(prev[1], 1)
                    if prev is not None:
                        if tb == 2:
                            norm_a()
                        elif tb == 3:
                            oT_prev = norm_b()
                        elif 5 <= tb <= 8:
                            emit_y(prev[0], oT_prev, tb - 5)
                    for fn in hooks.get(hp, {}).get(tb, []):
                        fn()
                prev = (hp, acc, exs)

            # ---- tail: last pass's deferred attn@V + normalize + y +
            # writeback, pipelined per j and per output block ----
            y_out = sb.tile([P, NSB, D], bf16, name="y_out")
            emit_av(prev[1], 14, prev[2].pop(14))
            emit_av(prev[1], 15, prev[2].pop(15))
            # denominator rows first so the reciprocal starts immediately
            nc.vector.tensor_copy(acc_sb[0:32, 0, :], prev[1][0:32, 0, :])
            norm_a(0)
            nc.vector.tensor_copy(acc_sb[0:32, 1, :], prev[1][0:32, 1, :])
            norm_a(1)
            # psum reads from base partition 32 are limited to 32 partitions
            nc.vector.tensor_copy(acc_sb[32:64, 0, :], prev[1][32:64, 0, :])
            nc.vector.tensor_copy(acc_sb[64:96, 0, :], prev[1][64:96, 0, :])
            nc.vector.tensor_copy(acc_sb[32:64, 1, :], prev[1][32:64, 1, :])
            nc.vector.tensor_copy(acc_sb[64:96, 1, :], prev[1][64:96, 1, :])
            oT_last = sb.tile([P, SPB], bf16, name="oTl", tag="oT", bufs=2)
            norm_b_j(0, oT_last, tail=True)
            norm_b_j(1, oT_last, tail=True)
            for sb4 in range(NSB):
                yps = [
                    ps.tile([P, 512], fp32, name="typs", tag=f"sc{sb4 % 2}",
                            bufs=1),
                    aux([P, 512]),
                ]
                for df in range(2):
                    nc.tensor.matmul(
                        yps[df][:], oT_last[:, ts(sb4, P)],
                        wo2_sb[:, H2 - 1, ts(df, 512)],
                        start=True, stop=True,
                    )
                for df in range(2):
                    nc.vector.tensor_add(
                        y_out[:, sb4, ts(df, 512)], yps[df][:],
                        y_sb[:, sb4, ts(df, 512)],
                    )
                # issue from the Scalar engine: idle after the last exp, and
                # far cheaper per issue than gpsimd
                nc.scalar.dma_start(y[ts(sb4, P), :], y_out[:, sb4, :])

    nc.compile()
    return nc


def make_in_maps(x, w_q, w_k, w_v, w_out):
    import ml_dtypes

    bf16 = ml_dtypes.bfloat16
    cst = np.zeros((P, 256), dtype=np.float32)
    cst[0:DK, 0:DK] = np.eye(DK, dtype=np.float32)
    cst[:, DK:192] = 1.0
    cstb = np.zeros((P, P), dtype=np.float32)
    cstb[0:DK, 0:DK] = np.eye(DK, dtype=np.float32)
    cstb[:, DK:P] = 1.0
    cstb = cstb.astype(bf16)
    x = np.ascontiguousarray(np.asarray(x, dtype=np.float32))
    w_q = np.asarray(w_q, dtype=np.float32)
    w_k = np.asarray(w_k, dtype=np.float32)
    w_v = np.asarray(w_v, dtype=np.float32)
    w_out = np.asarray(w_out, dtype=np.float32)

    wqT = np.ascontiguousarray(w_q.T.astype(bf16))
    wkT = np.ascontiguousarray(w_k.T.astype(bf16))
    wvT = np.ascontiguousarray(
        np.concatenate([w_v.T, np.zeros((D, 1), np.float32)], axis=1)
        .astype(bf16)
    )
    # head-pair-stacked w_out.T: wo2[phi*64+dv, hp, d] = w_out.T[(2hp+phi)*64+dv, d]
    wo2 = np.ascontiguousarray(
        w_out.T.reshape(H2, 2, DK, D).transpose(1, 2, 0, 3).reshape(P, H2, D)
        .astype(bf16)
    )

    in_maps = []
    for c in range(NCORES):
        b, r = divmod(c, GPB)
        # roll this core's query rows to the front; t-order is irrelevant
        # (attention sums over t), so K/V are unaffected
        xb = np.roll(x[b], -r * SPB, axis=0)
        xTc = np.ascontiguousarray(xb.T.astype(bf16))
        in_maps.append(
            {"xT": xTc, "wqT": wqT, "wkT": wkT, "wvT": wvT, "wo2": wo2,
             "cst": cst, "cstb": cstb}
        )
    return in_maps


_BUILD_CACHE = {}


def _cached_nc(scale: float):
    key = round(float(scale), 12)
    if key not in _BUILD_CACHE:
        _BUILD_CACHE[key] = build_bass(float(scale))
    return _BUILD_CACHE[key]


def run_on_hw(in_maps, scale, trace=False):
    from concourse.bass_utils import run_bass_kernel_spmd

    nc = _cached_nc(scale)
    return run_bass_kernel_spmd(nc, in_maps, list(range(NCORES)), trace=trace)


def assemble(results):
    out = np.empty((B, S, D), dtype=np.float32)
    for c in range(NCORES):
        b, r = divmod(c, GPB)
        out[b, r * SPB:(r + 1) * SPB] = results[c]["y"].astype(np.float32)
    return out


def kernel(x, w_q, w_k, w_v, w_out, softmax_scale):
    scale = float(np.asarray(softmax_scale).reshape(-1)[0])
    in_maps = make_in_maps(x, w_q, w_k, w_v, w_out)
    res = run_on_hw(in_maps, scale, trace=False)
    return assemble(res.results)


# revision 15
# speedup vs baseline: 1.0951x; 1.0951x over previous
"""Multi-head attention (multiquery K/V) Bass kernel for 8 trn2 NeuronCores.

Sharding: 8 cores = 2 batches x 4 query-row quarters. Each core computes the
full multiquery K/V projections for its batch (cheap, dk=64) and runs
attention + output projection for its 512 query rows over all 16 heads.
Output is a pure concatenation across cores -- no collectives.

Design (v5):
- Steady state is jointly PE/ACT bound: per t-block the PE runs 2 score +
  2 attn@V matmuls (512 cols, bf16) plus ~1 amortized projection matmul;
  the Scalar engine one exp over [128, 2, 512].
- Scores keep the full-128 contraction with the zero padding in the
  STATIONARY operand (K2Tz[:,0]=[K;0], K2Tz[:,1]=[0;K]); qz holds the head
  pair stacked [Qe;Qo] -- no big memzero, single evacuation per q-block.
- K and V projections fuse into one stationary ([w_k.T | w_v.T], 128 wide):
  one 8-matmul pass per x block yields K.T and V.T together, halving the
  projection matmuls of v3.  The attn@V denominator ones-column is
  memset-built, not DMA'd.
- attn@V lags exp by 2 t-blocks everywhere (ex bufs=4): the previous pass's
  last two attn@V land in tb0/tb1, the accumulator evacuates (split per j)
  at tb1, the new pass's attn@V starts at tb2.  No matmul burst at pass
  boundaries; hooks are spread so no t-block carries more than 2 extra
  matmuls.
- All input DMAs issue on the single gpsimd queue in exact first-use order
  (multiple queues just split HBM bandwidth and starve the critical path).
  Warm-up matmuls on a junk tile un-throttle the PE HAM and a dummy 1-col
  exp preloads the activation table set during the initial DMA wait.
- Tail: deferred attn@V right after the last exp; the reciprocal and the
  normalize multiplies read the accumulator PSUM directly (no evacuation);
  bc evacuation runs on the now-idle Scalar engine; junk matmuls keep the
  PE clock un-throttled through the normalize; per-block y matmuls ->
  single merged add -> DMA writeback pipeline.
"""

import sys

import numpy as np

if "/opt/trn_rl_repo" not in sys.path:
    sys.path.insert(0, "/opt/trn_rl_repo")

B, S, D = 2, 2048, 1024
H, DK = 16, 64
H2 = H // 2  # head pairs
P = 128
NCORES, GPB = 8, 4
SPB = S // GPB  # 512 query rows per core
KC = D // P  # 8 contraction subtiles over d_model
NT = S // P  # 16 key/t blocks
NSB = SPB // P  # 4 s blocks


def build_bass(scale: float, debug: bool = False):
    import concourse.bacc as bacc
    import concourse.mybir as mybir
    import concourse.tile as tile
    from concourse.bass import ts
    from concourse.dve_ops import (
        RECIP_APPROX_FAST_CONSTS,
        RECIPROCAL_APPROX_FAST,
    )

    fp32 = mybir.dt.float32
    mdt = mybir.dt.float32r
    Act = mybir.ActivationFunctionType

    bf16 = mybir.dt.bfloat16
    nc = bacc.Bacc(None, target_bir_lowering=False)
    xT = nc.dram_tensor("xT", [D, S], bf16, kind="ExternalInput")
    cstb = nc.dram_tensor("cstb", [P, 192], bf16, kind="ExternalInput")
    wqT = nc.dram_tensor("wqT", [D, D], bf16, kind="ExternalInput")
    wkv = nc.dram_tensor("wkv", [D, P], bf16, kind="ExternalInput")
    wo2 = nc.dram_tensor("wo2", [P, H2, D], bf16, kind="ExternalInput")
    y = nc.dram_tensor("y", [SPB, D], bf16, kind="ExternalOutput")

    xT3 = xT.rearrange("(po pi) s -> pi po s", pi=P)
    wq3 = wqT.rearrange("(po pi) d -> pi po d", pi=P)
    wkv3 = wkv.rearrange("(po pi) d -> pi po d", pi=P)

    with tile.TileContext(nc) as tc:
        with (
            tc.tile_pool(name="sb", bufs=1) as sb,
            tc.tile_pool(name="ps", bufs=1, space="PSUM") as ps,
        ):
            # ---- persistent SBUF ----
            cstb_sb = sb.tile([P, 192], bf16, name="cstb")
            K2Tz = sb.tile([P, 2, S], bf16, name="K2Tz")
            # Vp stationary: col 0 = denominator ones column, cols 32:96 = V'
            Vp = sb.tile([P, NT, 96], bf16, name="Vp")
            qz = sb.tile([P, KC, SPB], bf16, name="qz")
            rec1 = sb.tile([1, 2, SPB], mdt, name="rec1")
            rec1b = sb.tile([1, 2, SPB], bf16, name="rec1b")
            acc_sb = sb.tile([96, 2, SPB], fp32, name="acc_sb")
            y_sb = sb.tile([P, NSB, D], fp32, name="y_sb")
            wkv_sb = sb.tile([P, KC, P], bf16, name="wkv")
            wq_sb = sb.tile([P, KC, D], bf16, name="wq")
            wo2_sb = sb.tile([P, H2, D], bf16, name="wo2")
            xc0 = sb.tile([P, KC, SPB], bf16, name="xc0")
            junk = sb.tile([P, SPB], bf16, name="junk")
            dead = sb.tile([P, 8], fp32, name="dead")

            ident = cstb_sb[0:DK, 0:DK]
            ones1 = cstb_sb[0:1, DK:DK + 96]  # [1, 96] of ones (bf16)

            def aux(shape, dtype=fp32):
                return ps.tile(shape, dtype, name="aux", tag="aux", bufs=2)

            def junk_mm(n):
                # PE keep-warm: matmuls on the junk tile into a dead psum
                # slot so the HAM clock gate never sees a >3us idle window
                for _ in range(n):
                    wps = aux([P, SPB])
                    nc.tensor.matmul(wps[:], junk[:, 0:P], junk[:],
                                     start=True, stop=True)

            # ---- warm-up: junk matmuls un-throttle the PE HAM while DMAs
            # stream; a 1-col exp preloads the activation table set ----
            nc.vector.memset(junk[:], 0)
            junk_mm(4)

            # ---- input DMAs: single gpsimd queue, exact first-use order
            # (parallel queues would split HBM bandwidth and starve the
            # critical path) ----
            nc.gpsimd.dma_start(wkv_sb[:], wkv3[:])
            nc.gpsimd.dma_start(xc0[:, 0:4, :], xT3[:, 0:4, 0:SPB])
            nc.gpsimd.dma_start(wq_sb[:, :, ts(0, P)], wq3[:, :, ts(0, P)])
            nc.gpsimd.dma_start(xc0[:, 4:KC, :], xT3[:, 4:KC, 0:SPB])
            nc.gpsimd.dma_start(wq_sb[:, :, ts(1, P)], wq3[:, :, ts(1, P)])
            nc.gpsimd.dma_start(cstb_sb[:], cstb[:])
            xcs = {0: xc0}
            xc1 = sb.tile([P, KC, SPB], bf16, name="xc", tag="xc", bufs=2)
            nc.gpsimd.dma_start(xc1[:], xT3[:, :, ts(1, SPB)])
            xcs[1] = xc1
            xc2 = sb.tile([P, KC, SPB], bf16, name="xc", tag="xc", bufs=2)
            nc.gpsimd.dma_start(xc2[:], xT3[:, :, ts(2, SPB)])
            xcs[2] = xc2
            nc.gpsimd.dma_start(wo2_sb[:, 0, :], wo2[:, 0, :])
            xc3 = sb.tile([P, KC, SPB], bf16, name="xc", tag="xc", bufs=2)
            nc.gpsimd.dma_start(xc3[:], xT3[:, :, ts(3, SPB)])
            xcs[3] = xc3
            for i in range(2, KC):
                nc.gpsimd.dma_start(wo2_sb[:, i - 1, :], wo2[:, i - 1, :])
                nc.gpsimd.dma_start(wq_sb[:, :, ts(i, P)],
                                    wq3[:, :, ts(i, P)])
            nc.gpsimd.dma_start(wo2_sb[:, H2 - 1, :], wo2[:, H2 - 1, :])

            # act-table preload (reads zeroed junk; result never used)
            nc.scalar.activation(dead[:, 0:1], junk[:, 0:1], Act.Exp,
                                 scale=1.0)

            # ---- one-time constant regions (off the critical path) ----
            nc.vector.memset(K2Tz[DK:P, 0, :], 0)
            nc.vector.memset(K2Tz[0:DK, 1, :], 0)
            for tb in range(NT):
                # denominator ones column; col 1 is in the never-read pad
                # region (2-wide so the AP keeps a free dim)
                nc.vector.memset(Vp[:, tb, 0:2], 1.0)
            # fp32 tile (MEMSET rejects float32r) bitcast to fp32r at use
            ones_m = sb.tile([1, 96], fp32, name="ones_m")
            nc.vector.memset(ones_m[:], 1.0)

            # fused K/V projection emitters, split into <=1us pieces that
            # slot into per-tb PE slack; "a" starts the psum accumulation,
            # "b" finishes it and evacuates K.T / V.T
            pstate = {}

            def kv_a(c):
                kvps = aux([P, SPB])
                pstate[("kv", c)] = kvps
                for k in range(4):
                    nc.tensor.matmul(
                        kvps[:], wkv_sb[:, k, :], xcs[c][:, k, :],
                        start=(k == 0), stop=False,
                    )

            def kv_b(c):
                kvps = pstate.pop(("kv", c))
                for k in range(4, KC):
                    nc.tensor.matmul(
                        kvps[:], wkv_sb[:, k, :], xcs[c][:, k, :],
                        start=False, stop=(k == KC - 1),
                    )
                nc.vector.tensor_copy(K2Tz[0:DK, 0, ts(c, 512)],
                                      kvps[0:DK, :])
                nc.vector.tensor_copy(K2Tz[DK:P, 1, ts(c, 512)],
                                      kvps[0:DK, :])
                vsb = sb.tile([DK, SPB], bf16, name="vsb", tag="vsb", bufs=1)
                pstate[("vsb", c)] = vsb
                nc.vector.tensor_copy(vsb[:], kvps[DK:P, :])

            def v_tr(c):
                # PE-transpose V.T -> V' [t, dv] into Vp cols 32:96
                vsb = pstate.pop(("vsb", c))
                for pair in range(2):
                    trs = [aux([P, DK], bf16), aux([P, DK], bf16)]
                    for q in range(2):
                        nc.tensor.transpose(
                            trs[q][:], vsb[:, ts(2 * pair + q, P)], ident
                        )
                    for q in range(2):
                        nc.vector.tensor_copy(
                            Vp[:, 4 * c + 2 * pair + q, 32:96], trs[q][:]
                        )

            def q_a(m):
                qps = aux([P, SPB])
                pstate[("q", m)] = qps
                for k in range(4):
                    nc.tensor.matmul(
                        qps[:], wq_sb[:, k, ts(m, P)], xc0[:, k, :],
                        start=(k == 0), stop=False,
                    )

            def q_b(m):
                qps = pstate.pop(("q", m))
                for k in range(4, KC):
                    nc.tensor.matmul(
                        qps[:], wq_sb[:, k, ts(m, P)], xc0[:, k, :],
                        start=False, stop=(k == KC - 1),
                    )
                nc.vector.tensor_copy(qz[:, m, :], qps[:])

            # ---- pre-pass: K/V for c0 and Q for m0, m1 ----
            kv_a(0)
            kv_b(0)
            q_a(0)
            q_b(0)
            q_a(1)
            q_b(1)
            v_tr(0)

            # ---- attention passes, one head pair each ----
            def emit_av(acc, tb, ex):
                first, last = (tb == 0), (tb == NT - 1)
                for j in range(2):
                    nc.tensor.matmul(
                        acc[:, j, :], Vp[:, tb, :], ex[:, j, :],
                        start=first, stop=last,
                    )

            def norm_a():
                # reciprocal of the denominator rows; the offset-dropping
                # custom op needs base partition 0 (acc_sb row 0), then a
                # tiny cast so the broadcast matmul streams bf16
                c = RECIP_APPROX_FAST_CONSTS
                nc.vector._custom_dve(
                    RECIPROCAL_APPROX_FAST,
                    out=rec1[0:1, :, :], in0=acc_sb[0:1, :, :],
                    s0=c["s0"], s1=c["s1"], imm2=c["imm2"],
                )
                nc.vector.tensor_copy(rec1b[:], rec1[:])

            def norm_b_j(j, oT):
                # broadcast 1/den across partitions via a K=1-contraction
                # ones-stationary matmul, then DVE multiplies write the
                # pair-stacked normalized oT.  32-row pieces: spans starting
                # at partition 32/96 may cover at most 32 partitions, and
                # both inputs must share a base partition.
                bc = aux([96, SPB])
                nc.tensor.matmul(
                    bc[:], ones1, rec1b[0:1, j, :], start=True, stop=True
                )
                bcs = sb.tile([96, SPB], fp32, name="bcs", tag="vsb", bufs=1)
                nc.vector.tensor_copy(bcs[:], bc[:])
                for hf in range(2):
                    nc.vector.tensor_mul(
                        oT[j * DK + hf * 32:j * DK + hf * 32 + 32, :],
                        acc_sb[32 + hf * 32:64 + hf * 32, j, :],
                        bcs[32 + hf * 32:64 + hf * 32, :],
                    )

            def emit_y1(prev_hp, oT, sb4, df):
                yps = aux([P, 512])
                nc.tensor.matmul(
                    yps[:], oT[:, ts(sb4, P)],
                    wo2_sb[:, prev_hp, ts(df, 512)],
                    start=True, stop=True,
                )
                if prev_hp == 0:
                    nc.vector.tensor_copy(y_sb[:, sb4, ts(df, 512)], yps[:])
                else:
                    nc.vector.tensor_add(
                        y_sb[:, sb4, ts(df, 512)], yps[:],
                        y_sb[:, sb4, ts(df, 512)],
                    )

            def q_p1(m, k):
                # single-matmul q-projection piece (chunk k of 8)
                if k == 0:
                    pstate[("q", m)] = aux([P, SPB])
                qps = pstate[("q", m)]
                nc.tensor.matmul(
                    qps[:], wq_sb[:, k, ts(m, P)], xc0[:, k, :],
                    start=(k == 0), stop=(k == KC - 1),
                )

            def q_fin(m):
                qps = pstate.pop(("q", m))
                nc.vector.tensor_copy(qz[:, m, :], qps[:])

            # hook schedule: {pass: {tb: [closures]}}.  Deadlines: K2Tz
            # block c by sc(4c); Vp block c by av(4c) (lag 2); qz m by the
            # next pass's sc(0).  v_tr(c) is emitted >=2 tb before av(4c)
            # so its PE transposes precede the dependent attn@V.
            hooks = {
                0: {1: [lambda: kv_a(1)], 2: [lambda: kv_b(1)],
                    4: [lambda: v_tr(1)],
                    5: [lambda: kv_a(2)], 6: [lambda: kv_b(2)],
                    8: [lambda: v_tr(2)],
                    9: [lambda: kv_a(3)], 10: [lambda: kv_b(3)],
                    12: [lambda: v_tr(3)]},
            }
            for p in range(1, 7):
                for i in range(4):
                    hooks.setdefault(p, {}).setdefault(12 + i, []).append(
                        lambda m=p + 1, i=i: (q_p1(m, 2 * i),
                                              q_p1(m, 2 * i + 1))
                    )
                hooks.setdefault(p, {}).setdefault(15, []).append(
                    lambda m=p + 1: q_fin(m)
                )

            prev = None  # (hp, acc, tail exs {14, 15})
            for hp in range(H2):
                acc = ps.tile([96, 2, SPB], fp32, name="acc", tag="acc",
                              bufs=1)
                exs = {}
                oT_prev = None
                for tb in range(NT):
                    sc = ps.tile([P, 2, SPB], fp32, name=f"sc{tb % 2}",
                                 tag=f"sc{tb % 2}", bufs=1)
                    for j in range(2):
                        nc.tensor.matmul(
                            sc[:, j, :], K2Tz[:, j, ts(tb, P)], qz[:, hp, :],
                            start=True, stop=True,
                        )
                    # attn@V lags exp by 2; the previous pass's last two
                    # attn@V land in tb0/tb1, its accumulator evacuates at
                    # tb1 (split per j) and this pass's attn@V starts at tb2
                    if tb == 0 and prev is not None:
                        emit_av(prev[1], 14, prev[2].pop(14))
                    if tb == 1 and prev is not None:
                        emit_av(prev[1], 15, prev[2].pop(15))
                    if tb >= 2:
                        emit_av(acc, tb - 2, exs.pop(tb - 2))
                    ex = sb.tile([P, 2, SPB], bf16, name="ex", tag="ex",
                                 bufs=4)
                    nc.scalar.activation(ex[:], sc[:], Act.Exp, scale=scale)
                    exs[tb] = ex
                    if tb == 1 and prev is not None:
                        nc.vector.tensor_copy(acc_sb[:, 0, :],
                                              prev[1][:, 0, :])
                        nc.vector.tensor_copy(acc_sb[:, 1, :],
                                              prev[1][:, 1, :])
                    if prev is not None:
                        if tb == 2:
                            norm_a()
                        elif tb == 3:
                            oT_prev = sb.tile([P, SPB], bf16, name="oT",
                                              tag="oT", bufs=2)
                            norm_b_j(0, oT_prev)
                        elif tb == 4:
                            norm_b_j(1, oT_prev)
                        elif tb == 5:
                            emit_y1(prev[0], oT_prev, 0, 0)
                            emit_y1(prev[0], oT_prev, 0, 1)
                        elif 6 <= tb <= 11:
                            emit_y1(prev[0], oT_prev,
                                    (tb - 4) // 2, (tb - 4) % 2)
                    for fn in hooks.get(hp, {}).get(tb, []):
                        fn()
                prev = (hp, acc, exs)

            # ---- tail: last pass's deferred attn@V + normalize + y +
            # writeback.  The reciprocal and normalize multiplies read the
            # accumulator PSUM directly -- no evacuation; junk matmuls keep
            # the PE clock warm through the DVE chain ----
            y_out = sb.tile([P, NSB, D], bf16, name="y_out")
            pacc = prev[1]
            emit_av(pacc, 14, prev[2].pop(14))
            emit_av(pacc, 15, prev[2].pop(15))
            c = RECIP_APPROX_FAST_CONSTS
            nc.vector._custom_dve(
                RECIPROCAL_APPROX_FAST,
                out=rec1[0:1, :, :], in0=pacc[0:1, :, :],
                s0=c["s0"], s1=c["s1"], imm2=c["imm2"],
            )
            oT_last = sb.tile([P, SPB], bf16, name="oTl", tag="oT", bufs=2)
            bcl = {}
            for j in range(2):
                # fp32r moving direct from rec1: slower matmul than bf16 but
                # skips the cast on the tail critical path
                bcl[j] = aux([96, SPB])
                nc.tensor.matmul(
                    bcl[j][:], ones_m[:].bitcast(mdt), rec1[0:1, j, :],
                    start=True, stop=True
                )
            junk_mm(8)
            bcsl = {}
            for j in range(2):
                bcsl[j] = sb.tile([96, SPB], fp32, name="bcsl", tag="bcsl",
                                  bufs=2)
                nc.scalar.copy(bcsl[j][:], bcl[j][:])
            for j in range(2):
                for hf in range(2):
                    lo = 32 + hf * 32
                    nc.vector.tensor_mul(
                        oT_last[j * DK + hf * 32:j * DK + hf * 32 + 32, :],
                        pacc[lo:lo + 32, j, :],
                        bcsl[j][lo:lo + 32, :],
                    )
            for sb4 in range(NSB):
                yps = ps.tile([P, D], fp32, name="typs", tag=f"sc{sb4 % 2}",
                              bufs=1)
                for df in range(2):
                    nc.tensor.matmul(
                        yps[:, ts(df, 512)], oT_last[:, ts(sb4, P)],
                        wo2_sb[:, H2 - 1, ts(df, 512)],
                        start=True, stop=True,
                    )
                nc.vector.tensor_add(
                    y_out[:, sb4, :], yps[:], y_sb[:, sb4, :]
                )
                # issue from the Scalar engine: idle after the last exp, and
                # far cheaper per issue than gpsimd
                nc.scalar.dma_start(y[ts(sb4, P), :], y_out[:, sb4, :])

    nc.compile()
    return nc


def make_in_maps(x, w_q, w_k, w_v, w_out):
    import ml_dtypes

    bf16 = ml_dtypes.bfloat16
    cstb = np.zeros((P, 192), dtype=np.float32)
    cstb[0:DK, 0:DK] = np.eye(DK, dtype=np.float32)
    cstb[:, DK:192] = 1.0
    cstb = cstb.astype(bf16)
    x = np.ascontiguousarray(np.asarray(x, dtype=np.float32))
    w_q = np.asarray(w_q, dtype=np.float32)
    w_k = np.asarray(w_k, dtype=np.float32)
    w_v = np.asarray(w_v, dtype=np.float32)
    w_out = np.asarray(w_out, dtype=np.float32)

    wqT = np.ascontiguousarray(w_q.T.astype(bf16))
    wkv = np.ascontiguousarray(
        np.concatenate([w_k.T, w_v.T], axis=1).astype(bf16)
    )
    # head-pair-stacked w_out.T: wo2[phi*64+dv, hp, d] = w_out.T[(2hp+phi)*64+dv, d]
    wo2 = np.ascontiguousarray(
        w_out.T.reshape(H2, 2, DK, D).transpose(1, 2, 0, 3).reshape(P, H2, D)
        .astype(bf16)
    )

    in_maps = []
    for c in range(NCORES):
        b, r = divmod(c, GPB)
        # roll this core's query rows to the front; t-order is irrelevant
        # (attention sums over t), so K/V are unaffected
        xb = np.roll(x[b], -r * SPB, axis=0)
        xTc = np.ascontiguousarray(xb.T.astype(bf16))
        in_maps.append(
            {"xT": xTc, "wqT": wqT, "wkv": wkv, "wo2": wo2, "cstb": cstb}
        )
    return in_maps


_BUILD_CACHE = {}


def _cached_nc(scale: float):
    key = round(float(scale), 12)
    if key not in _BUILD_CACHE:
        _BUILD_CACHE[key] = build_bass(float(scale))
    return _BUILD_CACHE[key]


def run_on_hw(in_maps, scale, trace=False):
    from concourse.bass_utils import run_bass_kernel_spmd

    nc = _cached_nc(scale)
    return run_bass_kernel_spmd(nc, in_maps, list(range(NCORES)), trace=trace)


def assemble(results):
    out = np.empty((B, S, D), dtype=np.float32)
    for c in range(NCORES):
        b, r = divmod(c, GPB)
        out[b, r * SPB:(r + 1) * SPB] = results[c]["y"].astype(np.float32)
    return out


def kernel(x, w_q, w_k, w_v, w_out, softmax_scale):
    scale = float(np.asarray(softmax_scale).reshape(-1)[0])
    in_maps = make_in_maps(x, w_q, w_k, w_v, w_out)
    res = run_on_hw(in_maps, scale, trace=False)
    return assemble(res.results)


# revision 23
# speedup vs baseline: 1.1194x; 1.0222x over previous
"""Multi-head attention (multiquery K/V) Bass kernel for 8 trn2 NeuronCores.

Sharding: 8 cores = 2 batches x 4 query-row quarters. Each core computes the
full multiquery K/V projections for its batch (cheap, dk=64) and runs
attention + output projection for its 512 query rows over all 16 heads.
Output is a pure concatenation across cores -- no collectives.

Design (v5):
- Steady state is jointly PE/ACT bound: per t-block the PE runs 2 score +
  2 attn@V matmuls (512 cols, bf16) plus ~1 amortized projection matmul;
  the Scalar engine one exp over [128, 2, 512].
- Scores keep the full-128 contraction with the zero padding in the
  STATIONARY operand (K2Tz[:,0]=[K;0], K2Tz[:,1]=[0;K]); qz holds the head
  pair stacked [Qe;Qo] -- no big memzero, single evacuation per q-block.
- K and V projections fuse into one stationary ([w_k.T | w_v.T], 128 wide):
  one 8-matmul pass per x block yields K.T and V.T together, halving the
  projection matmuls of v3.  The attn@V denominator ones-column is
  memset-built, not DMA'd.
- attn@V lags exp by 2 t-blocks everywhere (ex bufs=4): the previous pass's
  last two attn@V land in tb0/tb1, the accumulator evacuates (split per j)
  at tb1, the new pass's attn@V starts at tb2.  No matmul burst at pass
  boundaries; hooks are spread so no t-block carries more than 2 extra
  matmuls.
- All input DMAs issue on the single gpsimd queue in exact first-use order
  (multiple queues just split HBM bandwidth and starve the critical path).
  Warm-up matmuls on a junk tile un-throttle the PE HAM and a dummy 1-col
  exp preloads the activation table set during the initial DMA wait.
- Tail: deferred attn@V right after the last exp; the reciprocal and the
  normalize multiplies read the accumulator PSUM directly (no evacuation);
  bc evacuation runs on the now-idle Scalar engine; junk matmuls keep the
  PE clock un-throttled through the normalize; per-block y matmuls ->
  single merged add -> DMA writeback pipeline.
"""

import sys

import numpy as np

if "/opt/trn_rl_repo" not in sys.path:
    sys.path.insert(0, "/opt/trn_rl_repo")

B, S, D = 2, 2048, 1024
H, DK = 16, 64
H2 = H // 2  # head pairs
P = 128
NCORES, GPB = 8, 4
SPB = S // GPB  # 512 query rows per core
KC = D // P  # 8 contraction subtiles over d_model
NT = S // P  # 16 key/t blocks
NSB = SPB // P  # 4 s blocks


def build_bass(scale: float, debug: bool = False):
    import concourse.bacc as bacc
    import concourse.mybir as mybir
    import concourse.tile as tile
    from concourse.bass import ts
    from concourse.dve_ops import (
        RECIP_APPROX_FAST_CONSTS,
        RECIPROCAL_APPROX_FAST,
    )

    fp32 = mybir.dt.float32
    mdt = mybir.dt.float32r
    Act = mybir.ActivationFunctionType

    bf16 = mybir.dt.bfloat16
    nc = bacc.Bacc(None, target_bir_lowering=False)
    xT = nc.dram_tensor("xT", [D, S], bf16, kind="ExternalInput")
    cstb = nc.dram_tensor("cstb", [P, 192], bf16, kind="ExternalInput")
    wqT = nc.dram_tensor("wqT", [D, D], bf16, kind="ExternalInput")
    wkv = nc.dram_tensor("wkv", [D, P], bf16, kind="ExternalInput")
    wo2 = nc.dram_tensor("wo2", [P, H2, D], bf16, kind="ExternalInput")
    y = nc.dram_tensor("y", [SPB, D], bf16, kind="ExternalOutput")

    xT3 = xT.rearrange("(po pi) s -> pi po s", pi=P)
    wq3 = wqT.rearrange("(po pi) d -> pi po d", pi=P)
    wkv3 = wkv.rearrange("(po pi) d -> pi po d", pi=P)

    with tile.TileContext(nc) as tc:
        with (
            tc.tile_pool(name="sb", bufs=1) as sb,
            tc.tile_pool(name="ps", bufs=1, space="PSUM") as ps,
        ):
            # ---- persistent SBUF ----
            cstb_sb = sb.tile([P, 192], bf16, name="cstb")
            K2Tz = sb.tile([P, 2, S], bf16, name="K2Tz")
            # Vp stationary: col 0 = denominator ones column, cols 32:96 = V'
            Vp = sb.tile([P, NT, 96], bf16, name="Vp")
            qz = sb.tile([P, KC, SPB], bf16, name="qz")
            rec1 = sb.tile([1, 2, SPB], mdt, name="rec1")
            rec1b = sb.tile([1, 2, SPB], bf16, name="rec1b")
            acc_sb = sb.tile([96, 2, SPB], fp32, name="acc_sb")
            y_sb = sb.tile([P, NSB, D], fp32, name="y_sb")
            wkv_sb = sb.tile([P, KC, P], bf16, name="wkv")
            wq_sb = sb.tile([P, KC, D], bf16, name="wq")
            wo2_sb = sb.tile([P, H2, D], bf16, name="wo2")
            xc0 = sb.tile([P, KC, SPB], bf16, name="xc0")
            junk = sb.tile([P, SPB], bf16, name="junk")
            dead = sb.tile([P, 8], fp32, name="dead")

            ident = cstb_sb[0:DK, 0:DK]
            ones1 = cstb_sb[0:1, DK:DK + 96]  # [1, 96] of ones (bf16)

            def aux(shape, dtype=fp32):
                return ps.tile(shape, dtype, name="aux", tag="aux", bufs=2)

            def junk_mm(n):
                # PE keep-warm: matmuls on the junk tile into a dead psum
                # slot so the HAM clock gate never sees a >3us idle window
                for _ in range(n):
                    wps = aux([P, SPB])
                    nc.tensor.matmul(wps[:], junk[:, 0:P], junk[:],
                                     start=True, stop=True)

            # ---- warm-up: junk matmuls un-throttle the PE HAM while DMAs
            # stream; a 1-col exp preloads the activation table set ----
            # enough junk matmuls that the PE HAM un-throttles (~3.4us of
            # sustained busy) right as the critical input DMAs land, so the
            # whole prepass chain runs at the warm 2.4 GHz clock
            nc.vector.memset(junk[:], 0)
            junk_mm(9)

            # ---- input DMAs: single gpsimd queue, exact first-use order
            # (parallel queues would split HBM bandwidth and starve the
            # critical path) ----
            nc.gpsimd.dma_start(wkv_sb[:], wkv3[:])
            nc.gpsimd.dma_start(xc0[:, 0:4, :], xT3[:, 0:4, 0:SPB])
            nc.gpsimd.dma_start(wq_sb[:, :, ts(0, P)], wq3[:, :, ts(0, P)])
            nc.gpsimd.dma_start(xc0[:, 4:KC, :], xT3[:, 4:KC, 0:SPB])
            nc.gpsimd.dma_start(wq_sb[:, :, ts(1, P)], wq3[:, :, ts(1, P)])
            nc.gpsimd.dma_start(cstb_sb[:], cstb[:])
            xcs = {0: xc0}
            xc1 = sb.tile([P, KC, SPB], bf16, name="xc", tag="xc", bufs=2)
            nc.gpsimd.dma_start(xc1[:], xT3[:, :, ts(1, SPB)])
            xcs[1] = xc1
            xc2 = sb.tile([P, KC, SPB], bf16, name="xc", tag="xc", bufs=2)
            nc.gpsimd.dma_start(xc2[:], xT3[:, :, ts(2, SPB)])
            xcs[2] = xc2
            nc.gpsimd.dma_start(wo2_sb[:, 0, :], wo2[:, 0, :])
            xc3 = sb.tile([P, KC, SPB], bf16, name="xc", tag="xc", bufs=2)
            nc.gpsimd.dma_start(xc3[:], xT3[:, :, ts(3, SPB)])
            xcs[3] = xc3
            for i in range(2, KC):
                nc.gpsimd.dma_start(wo2_sb[:, i - 1, :], wo2[:, i - 1, :])
                nc.gpsimd.dma_start(wq_sb[:, :, ts(i, P)],
                                    wq3[:, :, ts(i, P)])
            nc.gpsimd.dma_start(wo2_sb[:, H2 - 1, :], wo2[:, H2 - 1, :])

            # act-table preload (reads zeroed junk; result never used)
            nc.scalar.activation(dead[:, 0:1], junk[:, 0:1], Act.Exp,
                                 scale=1.0)

            # ---- one-time constant regions (off the critical path) ----
            nc.vector.memset(K2Tz[DK:P, 0, :], 0)
            nc.vector.memset(K2Tz[0:DK, 1, :], 0)
            for tb in range(NT):
                # denominator ones column; col 1 is in the never-read pad
                # region (2-wide so the AP keeps a free dim)
                nc.vector.memset(Vp[:, tb, 0:2], 1.0)
            # fp32 tile (MEMSET rejects float32r) bitcast to fp32r at use
            ones_m = sb.tile([1, 96], fp32, name="ones_m")
            nc.vector.memset(ones_m[:], 1.0)

            # fused K/V projection emitters, split into <=1us pieces that
            # slot into per-tb PE slack; "a" starts the psum accumulation,
            # "b" finishes it and evacuates K.T / V.T
            pstate = {}

            def kv_a(c):
                kvps = aux([P, SPB])
                pstate[("kv", c)] = kvps
                for k in range(4):
                    nc.tensor.matmul(
                        kvps[:], wkv_sb[:, k, :], xcs[c][:, k, :],
                        start=(k == 0), stop=False,
                    )

            def kv_b(c):
                kvps = pstate.pop(("kv", c))
                for k in range(4, KC):
                    nc.tensor.matmul(
                        kvps[:], wkv_sb[:, k, :], xcs[c][:, k, :],
                        start=False, stop=(k == KC - 1),
                    )
                nc.vector.tensor_copy(K2Tz[0:DK, 0, ts(c, 512)],
                                      kvps[0:DK, :])
                nc.vector.tensor_copy(K2Tz[DK:P, 1, ts(c, 512)],
                                      kvps[0:DK, :])
                vsb = sb.tile([DK, SPB], bf16, name="vsb", tag="vsb", bufs=1)
                pstate[("vsb", c)] = vsb
                nc.vector.tensor_copy(vsb[:], kvps[DK:P, :])

            def v_tr(c):
                # PE-transpose V.T -> V' [t, dv] into Vp cols 32:96
                vsb = pstate.pop(("vsb", c))
                for pair in range(2):
                    trs = [aux([P, DK], bf16), aux([P, DK], bf16)]
                    for q in range(2):
                        nc.tensor.transpose(
                            trs[q][:], vsb[:, ts(2 * pair + q, P)], ident
                        )
                    for q in range(2):
                        nc.vector.tensor_copy(
                            Vp[:, 4 * c + 2 * pair + q, 32:96], trs[q][:]
                        )

            def q_a(m):
                qps = aux([P, SPB])
                pstate[("q", m)] = qps
                for k in range(4):
                    nc.tensor.matmul(
                        qps[:], wq_sb[:, k, ts(m, P)], xc0[:, k, :],
                        start=(k == 0), stop=False,
                    )

            def q_b(m):
                qps = pstate.pop(("q", m))
                for k in range(4, KC):
                    nc.tensor.matmul(
                        qps[:], wq_sb[:, k, ts(m, P)], xc0[:, k, :],
                        start=False, stop=(k == KC - 1),
                    )
                nc.vector.tensor_copy(qz[:, m, :], qps[:])

            # ---- pre-pass: K/V for c0 and Q for m0, m1 ----
            kv_a(0)
            kv_b(0)
            q_a(0)
            q_b(0)
            q_a(1)
            q_b(1)
            v_tr(0)

            # ---- attention passes, one head pair each ----
            def emit_av(acc, tb, ex):
                first, last = (tb == 0), (tb == NT - 1)
                for j in range(2):
                    nc.tensor.matmul(
                        acc[:, j, :], Vp[:, tb, :], ex[:, j, :],
                        start=first, stop=last,
                    )

            def norm_a():
                # reciprocal of the denominator rows; the offset-dropping
                # custom op needs base partition 0 (acc_sb row 0), then a
                # tiny cast so the broadcast matmul streams bf16
                c = RECIP_APPROX_FAST_CONSTS
                nc.vector._custom_dve(
                    RECIPROCAL_APPROX_FAST,
                    out=rec1[0:1, :, :], in0=acc_sb[0:1, :, :],
                    s0=c["s0"], s1=c["s1"], imm2=c["imm2"],
                )
                nc.vector.tensor_copy(rec1b[:], rec1[:])

            def norm_b_j(j, oT):
                # broadcast 1/den across partitions via a K=1-contraction
                # ones-stationary matmul, then DVE multiplies write the
                # pair-stacked normalized oT.  32-row pieces: spans starting
                # at partition 32/96 may cover at most 32 partitions, and
                # both inputs must share a base partition.
                bc = aux([96, SPB])
                nc.tensor.matmul(
                    bc[:], ones1, rec1b[0:1, j, :], start=True, stop=True
                )
                bcs = sb.tile([96, SPB], fp32, name="bcs", tag="vsb", bufs=1)
                nc.vector.tensor_copy(bcs[:], bc[:])
                for hf in range(2):
                    nc.vector.tensor_mul(
                        oT[j * DK + hf * 32:j * DK + hf * 32 + 32, :],
                        acc_sb[32 + hf * 32:64 + hf * 32, j, :],
                        bcs[32 + hf * 32:64 + hf * 32, :],
                    )

            def emit_y1(prev_hp, oT, sb4, df):
                yps = aux([P, 512])
                nc.tensor.matmul(
                    yps[:], oT[:, ts(sb4, P)],
                    wo2_sb[:, prev_hp, ts(df, 512)],
                    start=True, stop=True,
                )
                if prev_hp == 0:
                    nc.vector.tensor_copy(y_sb[:, sb4, ts(df, 512)], yps[:])
                else:
                    nc.vector.tensor_add(
                        y_sb[:, sb4, ts(df, 512)], yps[:],
                        y_sb[:, sb4, ts(df, 512)],
                    )

            def q_p1(m, k):
                # single-matmul q-projection piece (chunk k of 8)
                if k == 0:
                    pstate[("q", m)] = aux([P, SPB])
                qps = pstate[("q", m)]
                nc.tensor.matmul(
                    qps[:], wq_sb[:, k, ts(m, P)], xc0[:, k, :],
                    start=(k == 0), stop=(k == KC - 1),
                )

            def q_fin(m):
                qps = pstate.pop(("q", m))
                nc.vector.tensor_copy(qz[:, m, :], qps[:])

            # hook schedule: {pass: {tb: [closures]}}.  Deadlines: K2Tz
            # block c by sc(4c); Vp block c by av(4c) (lag 2); qz m by the
            # next pass's sc(0).  v_tr(c) is emitted >=2 tb before av(4c)
            # so its PE transposes precede the dependent attn@V.
            hooks = {
                0: {1: [lambda: kv_a(1)], 2: [lambda: kv_b(1)],
                    4: [lambda: v_tr(1)],
                    5: [lambda: kv_a(2)], 6: [lambda: kv_b(2)],
                    8: [lambda: v_tr(2)],
                    9: [lambda: kv_a(3)], 10: [lambda: kv_b(3)],
                    12: [lambda: v_tr(3)]},
            }
            # q pieces end at tb14 so the qz evacuation lands well before
            # the next pass's first score matmul needs it
            for p in range(1, 7):
                for i in range(4):
                    hooks.setdefault(p, {}).setdefault(11 + i, []).append(
                        lambda m=p + 1, i=i: (q_p1(m, 2 * i),
                                              q_p1(m, 2 * i + 1))
                    )
                hooks.setdefault(p, {}).setdefault(14, []).append(
                    lambda m=p + 1: q_fin(m)
                )

            prev = None  # (hp, acc, tail exs {14, 15})
            for hp in range(H2):
                acc = ps.tile([96, 2, SPB], fp32, name="acc", tag="acc",
                              bufs=1)
                exs = {}
                oT_prev = None
                for tb in range(NT):
                    sc = ps.tile([P, 2, SPB], fp32, name=f"sc{tb % 2}",
                                 tag=f"sc{tb % 2}", bufs=1)
                    for j in range(2):
                        nc.tensor.matmul(
                            sc[:, j, :], K2Tz[:, j, ts(tb, P)], qz[:, hp, :],
                            start=True, stop=True,
                        )
                    # attn@V lags exp by 2; the previous pass's last two
                    # attn@V land in tb0/tb1, its accumulator evacuates at
                    # tb1 (split per j) and this pass's attn@V starts at tb2
                    if tb == 0 and prev is not None:
                        emit_av(prev[1], 14, prev[2].pop(14))
                    if tb == 1 and prev is not None:
                        emit_av(prev[1], 15, prev[2].pop(15))
                    if tb >= 2:
                        emit_av(acc, tb - 2, exs.pop(tb - 2))
                    ex = sb.tile([P, 2, SPB], bf16, name="ex", tag="ex",
                                 bufs=4)
                    nc.scalar.activation(ex[:], sc[:], Act.Exp, scale=scale)
                    exs[tb] = ex
                    if tb == 1 and prev is not None:
                        nc.vector.tensor_copy(acc_sb[:, 0, :],
                                              prev[1][:, 0, :])
                        nc.vector.tensor_copy(acc_sb[:, 1, :],
                                              prev[1][:, 1, :])
                    if prev is not None:
                        if tb == 2:
                            norm_a()
                        elif tb == 3:
                            oT_prev = sb.tile([P, SPB], bf16, name="oT",
                                              tag="oT", bufs=2)
                            norm_b_j(0, oT_prev)
                        elif tb == 4:
                            norm_b_j(1, oT_prev)
                        elif tb == 5:
                            emit_y1(prev[0], oT_prev, 0, 0)
                            emit_y1(prev[0], oT_prev, 0, 1)
                        elif 6 <= tb <= 9:
                            emit_y1(prev[0], oT_prev,
                                    (tb - 4) // 2, (tb - 4) % 2)
                        elif tb == 10:
                            emit_y1(prev[0], oT_prev, 3, 0)
                            emit_y1(prev[0], oT_prev, 3, 1)
                    for fn in hooks.get(hp, {}).get(tb, []):
                        fn()
                prev = (hp, acc, exs)

            # ---- tail: last pass's deferred attn@V + normalize + y +
            # writeback.  The reciprocal and normalize multiplies read the
            # accumulator PSUM directly -- no evacuation; junk matmuls keep
            # the PE clock warm through the DVE chain ----
            y_out = sb.tile([P, NSB, D], bf16, name="y_out")
            pacc = prev[1]
            emit_av(pacc, 14, prev[2].pop(14))
            emit_av(pacc, 15, prev[2].pop(15))
            c = RECIP_APPROX_FAST_CONSTS
            oT_last = sb.tile([P, SPB], bf16, name="oTl", tag="oT", bufs=2)
            bcl, bcsl = {}, {}
            # single reciprocal call over both j: the custom DVE op drops AP
            # offsets on HW, so any slice not at the tile base reads/writes
            # the wrong location (a per-j split raced and corrupted j0)
            nc.vector._custom_dve(
                RECIPROCAL_APPROX_FAST,
                out=rec1[0:1, :, :], in0=pacc[0:1, :, :],
                s0=c["s0"], s1=c["s1"], imm2=c["imm2"],
            )
            for j in range(2):
                # fp32r moving direct from rec1 skips a cast
                bcl[j] = aux([96, SPB])
                nc.tensor.matmul(
                    bcl[j][:], ones_m[:].bitcast(mdt), rec1[0:1, j, :],
                    start=True, stop=True
                )
                bcsl[j] = sb.tile([96, SPB], fp32, name="bcsl", tag="bcsl",
                                  bufs=2)
                nc.scalar.copy(bcsl[j][:], bcl[j][:])
            junk_mm(8)
            for j in range(2):
                for hf in range(2):
                    lo = 32 + hf * 32
                    nc.vector.tensor_mul(
                        oT_last[j * DK + hf * 32:j * DK + hf * 32 + 32, :],
                        pacc[lo:lo + 32, j, :],
                        bcsl[j][lo:lo + 32, :],
                    )
            for sb4 in range(NSB):
                yps = ps.tile([P, D], fp32, name="typs", tag=f"sc{sb4 % 2}",
                              bufs=1)
                for df in range(2):
                    nc.tensor.matmul(
                        yps[:, ts(df, 512)], oT_last[:, ts(sb4, P)],
                        wo2_sb[:, H2 - 1, ts(df, 512)],
                        start=True, stop=True,
                    )
                nc.vector.tensor_add(
                    y_out[:, sb4, :], yps[:], y_sb[:, sb4, :]
                )
                # issue from the Scalar engine: idle after the last exp, and
                # far cheaper per issue than gpsimd
                nc.scalar.dma_start(y[ts(sb4, P), :], y_out[:, sb4, :])

    nc.compile()
    return nc


def make_in_maps(x, w_q, w_k, w_v, w_out):
    import ml_dtypes

    bf16 = ml_dtypes.bfloat16
    cstb = np.zeros((P, 192), dtype=np.float32)
    cstb[0:DK, 0:DK] = np.eye(DK, dtype=np.float32)
    cstb[:, DK:192] = 1.0
    cstb = cstb.astype(bf16)
    x = np.ascontiguousarray(np.asarray(x, dtype=np.float32))
    w_q = np.asarray(w_q, dtype=np.float32)
    w_k = np.asarray(w_k, dtype=np.float32)
    w_v = np.asarray(w_v, dtype=np.float32)
    w_out = np.asarray(w_out, dtype=np.float32)

    wqT = np.ascontiguousarray(w_q.T.astype(bf16))
    wkv = np.ascontiguousarray(
        np.concatenate([w_k.T, w_v.T], axis=1).astype(bf16)
    )
    # head-pair-stacked w_out.T: wo2[phi*64+dv, hp, d] = w_out.T[(2hp+phi)*64+dv, d]
    wo2 = np.ascontiguousarray(
        w_out.T.reshape(H2, 2, DK, D).transpose(1, 2, 0, 3).reshape(P, H2, D)
        .astype(bf16)
    )

    in_maps = []
    for c in range(NCORES):
        b, r = divmod(c, GPB)
        # roll this core's query rows to the front; t-order is irrelevant
        # (attention sums over t), so K/V are unaffected
        xb = np.roll(x[b], -r * SPB, axis=0)
        xTc = np.ascontiguousarray(xb.T.astype(bf16))
        in_maps.append(
            {"xT": xTc, "wqT": wqT, "wkv": wkv, "wo2": wo2, "cstb": cstb}
        )
    return in_maps


_BUILD_CACHE = {}


def _cached_nc(scale: float):
    key = round(float(scale), 12)
    if key not in _BUILD_CACHE:
        _BUILD_CACHE[key] = build_bass(float(scale))
    return _BUILD_CACHE[key]


def run_on_hw(in_maps, scale, trace=False):
    from concourse.bass_utils import run_bass_kernel_spmd

    nc = _cached_nc(scale)
    return run_bass_kernel_spmd(nc, in_maps, list(range(NCORES)), trace=trace)


def assemble(results):
    out = np.empty((B, S, D), dtype=np.float32)
    for c in range(NCORES):
        b, r = divmod(c, GPB)
        out[b, r * SPB:(r + 1) * SPB] = results[c]["y"].astype(np.float32)
    return out


def kernel(x, w_q, w_k, w_v, w_out, softmax_scale):
    scale = float(np.asarray(softmax_scale).reshape(-1)[0])
    in_maps = make_in_maps(x, w_q, w_k, w_v, w_out)
    res = run_on_hw(in_maps, scale, trace=False)
    return assemble(res.results)


# revision 31
# speedup vs baseline: 1.1202x; 1.0007x over previous
"""Multi-head attention (multiquery K/V) Bass kernel for 8 trn2 NeuronCores.

Sharding: 8 cores = 2 batches x 4 query-row quarters. Each core computes the
full multiquery K/V projections for its batch (cheap, dk=64) and runs
attention + output projection for its 512 query rows over all 16 heads.
Output is a pure concatenation across cores -- no collectives.

Design (v5):
- Steady state is jointly PE/ACT bound: per t-block the PE runs 2 score +
  2 attn@V matmuls (512 cols, bf16) plus ~1 amortized projection matmul;
  the Scalar engine one exp over [128, 2, 512].
- Scores keep the full-128 contraction with the zero padding in the
  STATIONARY operand (K2Tz[:,0]=[K;0], K2Tz[:,1]=[0;K]); qz holds the head
  pair stacked [Qe;Qo] -- no big memzero, single evacuation per q-block.
- K and V projections fuse into one stationary ([w_k.T | w_v.T], 128 wide):
  one 8-matmul pass per x block yields K.T and V.T together, halving the
  projection matmuls of v3.  The attn@V denominator ones-column is
  memset-built, not DMA'd.
- attn@V lags exp by 2 t-blocks everywhere (ex bufs=4): the previous pass's
  last two attn@V land in tb0/tb1, the accumulator evacuates (split per j)
  at tb1, the new pass's attn@V starts at tb2.  No matmul burst at pass
  boundaries; hooks are spread so no t-block carries more than 2 extra
  matmuls.
- All input DMAs issue on the single gpsimd queue in exact first-use order
  (multiple queues just split HBM bandwidth and starve the critical path).
  Warm-up matmuls on a junk tile un-throttle the PE HAM and a dummy 1-col
  exp preloads the activation table set during the initial DMA wait.
- Tail: deferred attn@V right after the last exp; the reciprocal and the
  normalize multiplies read the accumulator PSUM directly (no evacuation);
  bc evacuation runs on the now-idle Scalar engine; junk matmuls keep the
  PE clock un-throttled through the normalize; per-block y matmuls ->
  single merged add -> DMA writeback pipeline.
"""

import sys

import numpy as np

if "/opt/trn_rl_repo" not in sys.path:
    sys.path.insert(0, "/opt/trn_rl_repo")

B, S, D = 2, 2048, 1024
H, DK = 16, 64
H2 = H // 2  # head pairs
P = 128
NCORES, GPB = 8, 4
SPB = S // GPB  # 512 query rows per core
KC = D // P  # 8 contraction subtiles over d_model
NT = S // P  # 16 key/t blocks
NSB = SPB // P  # 4 s blocks


def build_bass(scale: float, debug: bool = False):
    import concourse.bacc as bacc
    import concourse.mybir as mybir
    import concourse.tile as tile
    from concourse.bass import ts
    from concourse.dve_ops import (
        RECIP_APPROX_FAST_CONSTS,
        RECIPROCAL_APPROX_FAST,
    )

    fp32 = mybir.dt.float32
    mdt = mybir.dt.float32r
    Act = mybir.ActivationFunctionType

    bf16 = mybir.dt.bfloat16
    nc = bacc.Bacc(None, target_bir_lowering=False)
    xT = nc.dram_tensor("xT", [D, S], bf16, kind="ExternalInput")
    cstb = nc.dram_tensor("cstb", [P, 192], bf16, kind="ExternalInput")
    wqT = nc.dram_tensor("wqT", [D, D], bf16, kind="ExternalInput")
    wkv = nc.dram_tensor("wkv", [D, P], bf16, kind="ExternalInput")
    wo2 = nc.dram_tensor("wo2", [P, H2, D], bf16, kind="ExternalInput")
    y = nc.dram_tensor("y", [SPB, D], bf16, kind="ExternalOutput")

    xT3 = xT.rearrange("(po pi) s -> pi po s", pi=P)
    wq3 = wqT.rearrange("(po pi) d -> pi po d", pi=P)
    wkv3 = wkv.rearrange("(po pi) d -> pi po d", pi=P)

    with tile.TileContext(nc) as tc:
        with (
            tc.tile_pool(name="sb", bufs=1) as sb,
            tc.tile_pool(name="ps", bufs=1, space="PSUM") as ps,
        ):
            # ---- persistent SBUF ----
            cstb_sb = sb.tile([P, 192], bf16, name="cstb")
            K2Tz = sb.tile([P, 2, S], bf16, name="K2Tz")
            # Vp stationary: col 0 = denominator ones column, cols 32:96 = V'
            Vp = sb.tile([P, NT, 96], bf16, name="Vp")
            qz = sb.tile([P, KC, SPB], bf16, name="qz")
            rec1 = sb.tile([1, 2, SPB], mdt, name="rec1")
            rec1b = sb.tile([1, 2, SPB], bf16, name="rec1b")
            acc_sb = sb.tile([96, 2, SPB], fp32, name="acc_sb")
            y_sb = sb.tile([P, NSB, D], fp32, name="y_sb")
            wkv_sb = sb.tile([P, KC, P], bf16, name="wkv")
            wq_sb = sb.tile([P, KC, D], bf16, name="wq")
            wo2_sb = sb.tile([P, H2, D], bf16, name="wo2")
            xc0 = sb.tile([P, KC, SPB], bf16, name="xc0")
            junk = sb.tile([P, SPB], bf16, name="junk")
            dead = sb.tile([P, 8], fp32, name="dead")

            ident = cstb_sb[0:DK, 0:DK]
            ones1 = cstb_sb[0:1, DK:DK + 96]  # [1, 96] of ones (bf16)

            def aux(shape, dtype=fp32):
                return ps.tile(shape, dtype, name="aux", tag="aux", bufs=2)

            def junk_mm(n):
                # PE keep-warm: matmuls on the junk tile into a dead psum
                # slot so the HAM clock gate never sees a >3us idle window
                for _ in range(n):
                    wps = aux([P, SPB])
                    nc.tensor.matmul(wps[:], junk[:, 0:P], junk[:],
                                     start=True, stop=True)

            # ---- warm-up: junk matmuls un-throttle the PE HAM while DMAs
            # stream; a 1-col exp preloads the activation table set ----
            # enough junk matmuls that the PE HAM un-throttles (~3.4us of
            # sustained busy) right as the critical input DMAs land, so the
            # whole prepass chain runs at the warm 2.4 GHz clock
            nc.vector.memset(junk[:], 0)
            junk_mm(12)

            # ---- input DMAs: single gpsimd queue, exact first-use order
            # (parallel queues would split HBM bandwidth and starve the
            # critical path) ----
            nc.gpsimd.dma_start(wkv_sb[:], wkv3[:])
            nc.gpsimd.dma_start(xc0[:, 0:4, :], xT3[:, 0:4, 0:SPB])
            nc.gpsimd.dma_start(xc0[:, 4:KC, :], xT3[:, 4:KC, 0:SPB])
            nc.gpsimd.dma_start(wq_sb[:, :, ts(0, P)], wq3[:, :, ts(0, P)])
            nc.gpsimd.dma_start(wq_sb[:, :, ts(1, P)], wq3[:, :, ts(1, P)])
            nc.gpsimd.dma_start(cstb_sb[:], cstb[:])
            xcs = {0: xc0}
            xc1 = sb.tile([P, KC, SPB], bf16, name="xc", tag="xc", bufs=2)
            nc.gpsimd.dma_start(xc1[:], xT3[:, :, ts(1, SPB)])
            xcs[1] = xc1
            xc2 = sb.tile([P, KC, SPB], bf16, name="xc", tag="xc", bufs=2)
            nc.gpsimd.dma_start(xc2[:], xT3[:, :, ts(2, SPB)])
            xcs[2] = xc2
            nc.gpsimd.dma_start(wo2_sb[:, 0, :], wo2[:, 0, :])
            xc3 = sb.tile([P, KC, SPB], bf16, name="xc", tag="xc", bufs=2)
            nc.gpsimd.dma_start(xc3[:], xT3[:, :, ts(3, SPB)])
            xcs[3] = xc3
            for i in range(2, KC):
                nc.gpsimd.dma_start(wo2_sb[:, i - 1, :], wo2[:, i - 1, :])
                nc.gpsimd.dma_start(wq_sb[:, :, ts(i, P)],
                                    wq3[:, :, ts(i, P)])
            nc.gpsimd.dma_start(wo2_sb[:, H2 - 1, :], wo2[:, H2 - 1, :])

            # act-table preload (reads zeroed junk; result never used)
            nc.scalar.activation(dead[:, 0:1], junk[:, 0:1], Act.Exp,
                                 scale=1.0)

            # ---- one-time constant regions (off the critical path) ----
            nc.vector.memset(K2Tz[DK:P, 0, :], 0)
            nc.vector.memset(K2Tz[0:DK, 1, :], 0)
            for tb in range(NT):
                # denominator ones column; col 1 is in the never-read pad
                # region (2-wide so the AP keeps a free dim)
                nc.vector.memset(Vp[:, tb, 0:2], 1.0)
            # fp32 tile (MEMSET rejects float32r) bitcast to fp32r at use
            ones_m = sb.tile([1, 96], fp32, name="ones_m")
            nc.vector.memset(ones_m[:], 1.0)

            # fused K/V projection emitters, split into <=1us pieces that
            # slot into per-tb PE slack; "a" starts the psum accumulation,
            # "b" finishes it and evacuates K.T / V.T
            pstate = {}

            def kv_a(c):
                kvps = aux([P, SPB])
                pstate[("kv", c)] = kvps
                for k in range(4):
                    nc.tensor.matmul(
                        kvps[:], wkv_sb[:, k, :], xcs[c][:, k, :],
                        start=(k == 0), stop=False,
                    )

            def kv_b(c):
                kvps = pstate.pop(("kv", c))
                for k in range(4, KC):
                    nc.tensor.matmul(
                        kvps[:], wkv_sb[:, k, :], xcs[c][:, k, :],
                        start=False, stop=(k == KC - 1),
                    )
                nc.vector.tensor_copy(K2Tz[0:DK, 0, ts(c, 512)],
                                      kvps[0:DK, :])
                nc.vector.tensor_copy(K2Tz[DK:P, 1, ts(c, 512)],
                                      kvps[0:DK, :])
                vsb = sb.tile([DK, SPB], bf16, name="vsb", tag="vsb", bufs=1)
                pstate[("vsb", c)] = vsb
                nc.vector.tensor_copy(vsb[:], kvps[DK:P, :])

            def v_tr(c):
                # PE-transpose V.T -> V' [t, dv] into Vp cols 32:96
                vsb = pstate.pop(("vsb", c))
                for pair in range(2):
                    trs = [aux([P, DK], bf16), aux([P, DK], bf16)]
                    for q in range(2):
                        nc.tensor.transpose(
                            trs[q][:], vsb[:, ts(2 * pair + q, P)], ident
                        )
                    for q in range(2):
                        nc.vector.tensor_copy(
                            Vp[:, 4 * c + 2 * pair + q, 32:96], trs[q][:]
                        )

            def q_a(m):
                qps = aux([P, SPB])
                pstate[("q", m)] = qps
                for k in range(4):
                    nc.tensor.matmul(
                        qps[:], wq_sb[:, k, ts(m, P)], xc0[:, k, :],
                        start=(k == 0), stop=False,
                    )

            def q_b(m):
                qps = pstate.pop(("q", m))
                for k in range(4, KC):
                    nc.tensor.matmul(
                        qps[:], wq_sb[:, k, ts(m, P)], xc0[:, k, :],
                        start=False, stop=(k == KC - 1),
                    )
                nc.vector.tensor_copy(qz[:, m, :], qps[:])

            # ---- pre-pass: K/V for c0 and Q for m0, m1 ----
            kv_a(0)
            kv_b(0)
            q_a(0)
            q_b(0)
            q_a(1)
            q_b(1)
            v_tr(0)

            # ---- attention passes, one head pair each ----
            def emit_av(acc, tb, ex):
                first, last = (tb == 0), (tb == NT - 1)
                for j in range(2):
                    nc.tensor.matmul(
                        acc[:, j, :], Vp[:, tb, :], ex[:, j, :],
                        start=first, stop=last,
                    )

            def norm_a():
                # reciprocal of the denominator rows; the offset-dropping
                # custom op needs base partition 0 (acc_sb row 0), then a
                # tiny cast so the broadcast matmul streams bf16
                c = RECIP_APPROX_FAST_CONSTS
                nc.vector._custom_dve(
                    RECIPROCAL_APPROX_FAST,
                    out=rec1[0:1, :, :], in0=acc_sb[0:1, :, :],
                    s0=c["s0"], s1=c["s1"], imm2=c["imm2"],
                )
                nc.vector.tensor_copy(rec1b[:], rec1[:])

            def norm_b_j(j, oT):
                # broadcast 1/den across partitions via a K=1-contraction
                # ones-stationary matmul, then DVE multiplies write the
                # pair-stacked normalized oT.  32-row pieces: spans starting
                # at partition 32/96 may cover at most 32 partitions, and
                # both inputs must share a base partition.
                bc = aux([96, SPB])
                nc.tensor.matmul(
                    bc[:], ones1, rec1b[0:1, j, :], start=True, stop=True
                )
                bcs = sb.tile([96, SPB], fp32, name="bcs", tag="vsb", bufs=1)
                nc.vector.tensor_copy(bcs[:], bc[:])
                for hf in range(2):
                    nc.vector.tensor_mul(
                        oT[j * DK + hf * 32:j * DK + hf * 32 + 32, :],
                        acc_sb[32 + hf * 32:64 + hf * 32, j, :],
                        bcs[32 + hf * 32:64 + hf * 32, :],
                    )

            def emit_y1(prev_hp, oT, sb4, df):
                yps = aux([P, 512])
                nc.tensor.matmul(
                    yps[:], oT[:, ts(sb4, P)],
                    wo2_sb[:, prev_hp, ts(df, 512)],
                    start=True, stop=True,
                )
                if prev_hp == 0:
                    nc.vector.tensor_copy(y_sb[:, sb4, ts(df, 512)], yps[:])
                else:
                    nc.vector.tensor_add(
                        y_sb[:, sb4, ts(df, 512)], yps[:],
                        y_sb[:, sb4, ts(df, 512)],
                    )

            def q_p1(m, k):
                # single-matmul q-projection piece (chunk k of 8)
                if k == 0:
                    pstate[("q", m)] = aux([P, SPB])
                qps = pstate[("q", m)]
                nc.tensor.matmul(
                    qps[:], wq_sb[:, k, ts(m, P)], xc0[:, k, :],
                    start=(k == 0), stop=(k == KC - 1),
                )

            def q_fin(m):
                qps = pstate.pop(("q", m))
                nc.vector.tensor_copy(qz[:, m, :], qps[:])

            # hook schedule: {pass: {tb: [closures]}}.  Deadlines: K2Tz
            # block c by sc(4c); Vp block c by av(4c) (lag 2); qz m by the
            # next pass's sc(0).  v_tr(c) is emitted >=2 tb before av(4c)
            # so its PE transposes precede the dependent attn@V.
            hooks = {
                0: {1: [lambda: kv_a(1)], 2: [lambda: kv_b(1)],
                    4: [lambda: v_tr(1)],
                    5: [lambda: kv_a(2)], 6: [lambda: kv_b(2)],
                    8: [lambda: v_tr(2)],
                    9: [lambda: kv_a(3)], 10: [lambda: kv_b(3)],
                    12: [lambda: v_tr(3)]},
            }
            # q pieces at tb6-9 (after the bc matmuls release an aux slot)
            # so the qz evacuation lands 6 t-blocks before the next pass's
            # first score matmul needs it
            for p in range(1, 7):
                for i in range(4):
                    hooks.setdefault(p, {}).setdefault(6 + i, []).append(
                        lambda m=p + 1, i=i: (q_p1(m, 2 * i),
                                              q_p1(m, 2 * i + 1))
                    )
                hooks.setdefault(p, {}).setdefault(9, []).append(
                    lambda m=p + 1: q_fin(m)
                )

            prev = None  # (hp, acc, tail exs {14, 15})
            for hp in range(H2):
                acc = ps.tile([96, 2, SPB], fp32, name="acc", tag="acc",
                              bufs=1)
                exs = {}
                oT_prev = None
                for tb in range(NT):
                    sc = ps.tile([P, 2, SPB], fp32, name=f"sc{tb % 2}",
                                 tag=f"sc{tb % 2}", bufs=1)
                    for j in range(2):
                        nc.tensor.matmul(
                            sc[:, j, :], K2Tz[:, j, ts(tb, P)], qz[:, hp, :],
                            start=True, stop=True,
                        )
                    # attn@V lags exp by 2; the previous pass's last two
                    # attn@V land in tb0/tb1, its accumulator evacuates at
                    # tb1 (split per j) and this pass's attn@V starts at tb2
                    if tb == 0 and prev is not None:
                        emit_av(prev[1], 14, prev[2].pop(14))
                    if tb == 1 and prev is not None:
                        emit_av(prev[1], 15, prev[2].pop(15))
                    if tb >= 2:
                        emit_av(acc, tb - 2, exs.pop(tb - 2))
                    ex = sb.tile([P, 2, SPB], bf16, name="ex", tag="ex",
                                 bufs=4)
                    nc.scalar.activation(ex[:], sc[:], Act.Exp, scale=scale)
                    exs[tb] = ex
                    if tb == 1 and prev is not None:
                        nc.vector.tensor_copy(acc_sb[:, 0, :],
                                              prev[1][:, 0, :])
                        nc.vector.tensor_copy(acc_sb[:, 1, :],
                                              prev[1][:, 1, :])
                    if prev is not None:
                        # norm/y positions chosen so no PE instruction ever
                        # waits on the DVE normalize chain: bc j0 fires at
                        # tb4 (cast done ~tb3.8), y(0,0) at tb10 (oT done
                        # ~tb7.5); tb12-15 stay light for the pass boundary
                        if tb == 2:
                            norm_a()
                        elif tb == 4:
                            oT_prev = sb.tile([P, SPB], bf16, name="oT",
                                              tag="oT", bufs=2)
                            norm_b_j(0, oT_prev)
                        elif tb == 5:
                            norm_b_j(1, oT_prev)
                        elif tb == 10:
                            emit_y1(prev[0], oT_prev, 0, 0)
                            emit_y1(prev[0], oT_prev, 0, 1)
                        elif tb == 11:
                            emit_y1(prev[0], oT_prev, 1, 0)
                            emit_y1(prev[0], oT_prev, 1, 1)
                        elif tb >= 12:
                            emit_y1(prev[0], oT_prev,
                                    (tb - 8) // 2, (tb - 8) % 2)
                    for fn in hooks.get(hp, {}).get(tb, []):
                        fn()
                prev = (hp, acc, exs)

            # ---- tail: last pass's deferred attn@V + normalize + y +
            # writeback.  The reciprocal and normalize multiplies read the
            # accumulator PSUM directly -- no evacuation; junk matmuls keep
            # the PE clock warm through the DVE chain ----
            y_out = sb.tile([P, NSB, D], bf16, name="y_out")
            pacc = prev[1]
            emit_av(pacc, 14, prev[2].pop(14))
            emit_av(pacc, 15, prev[2].pop(15))
            c = RECIP_APPROX_FAST_CONSTS
            oT_last = sb.tile([P, SPB], bf16, name="oTl", tag="oT", bufs=2)
            bcl, bcsl = {}, {}
            # single reciprocal call over both j: the custom DVE op drops AP
            # offsets on HW, so any slice not at the tile base reads/writes
            # the wrong location (a per-j split raced and corrupted j0)
            nc.vector._custom_dve(
                RECIPROCAL_APPROX_FAST,
                out=rec1[0:1, :, :], in0=pacc[0:1, :, :],
                s0=c["s0"], s1=c["s1"], imm2=c["imm2"],
            )
            for j in range(2):
                # fp32r moving direct from rec1 skips a cast
                bcl[j] = aux([96, SPB])
                nc.tensor.matmul(
                    bcl[j][:], ones_m[:].bitcast(mdt), rec1[0:1, j, :],
                    start=True, stop=True
                )
                bcsl[j] = sb.tile([96, SPB], fp32, name="bcsl", tag="bcsl",
                                  bufs=2)
                nc.scalar.copy(bcsl[j][:], bcl[j][:])
            junk_mm(8)
            for j in range(2):
                for hf in range(2):
                    lo = 32 + hf * 32
                    nc.vector.tensor_mul(
                        oT_last[j * DK + hf * 32:j * DK + hf * 32 + 32, :],
                        pacc[lo:lo + 32, j, :],
                        bcsl[j][lo:lo + 32, :],
                    )
            for sb4 in range(NSB):
                yps = ps.tile([P, D], fp32, name="typs", tag=f"sc{sb4 % 2}",
                              bufs=1)
                for df in range(2):
                    nc.tensor.matmul(
                        yps[:, ts(df, 512)], oT_last[:, ts(sb4, P)],
                        wo2_sb[:, H2 - 1, ts(df, 512)],
                        start=True, stop=True,
                    )
                nc.vector.tensor_add(
                    y_out[:, sb4, :], yps[:], y_sb[:, sb4, :]
                )
                # issue from the Scalar engine: idle after the last exp, and
                # far cheaper per issue than gpsimd
                nc.scalar.dma_start(y[ts(sb4, P), :], y_out[:, sb4, :])

    nc.compile()
    return nc


def make_in_maps(x, w_q, w_k, w_v, w_out):
    import ml_dtypes

    bf16 = ml_dtypes.bfloat16
    cstb = np.zeros((P, 192), dtype=np.float32)
    cstb[0:DK, 0:DK] = np.eye(DK, dtype=np.float32)
    cstb[:, DK:192] = 1.0
    cstb = cstb.astype(bf16)
    x = np.ascontiguousarray(np.asarray(x, dtype=np.float32))
    w_q = np.asarray(w_q, dtype=np.float32)
    w_k = np.asarray(w_k, dtype=np.float32)
    w_v = np.asarray(w_v, dtype=np.float32)
    w_out = np.asarray(w_out, dtype=np.float32)

    wqT = np.ascontiguousarray(w_q.T.astype(bf16))
    wkv = np.ascontiguousarray(
        np.concatenate([w_k.T, w_v.T], axis=1).astype(bf16)
    )
    # head-pair-stacked w_out.T: wo2[phi*64+dv, hp, d] = w_out.T[(2hp+phi)*64+dv, d]
    wo2 = np.ascontiguousarray(
        w_out.T.reshape(H2, 2, DK, D).transpose(1, 2, 0, 3).reshape(P, H2, D)
        .astype(bf16)
    )

    in_maps = []
    for c in range(NCORES):
        b, r = divmod(c, GPB)
        # roll this core's query rows to the front; t-order is irrelevant
        # (attention sums over t), so K/V are unaffected
        xb = np.roll(x[b], -r * SPB, axis=0)
        xTc = np.ascontiguousarray(xb.T.astype(bf16))
        in_maps.append(
            {"xT": xTc, "wqT": wqT, "wkv": wkv, "wo2": wo2, "cstb": cstb}
        )
    return in_maps


_BUILD_CACHE = {}


def _cached_nc(scale: float):
    key = round(float(scale), 12)
    if key not in _BUILD_CACHE:
        _BUILD_CACHE[key] = build_bass(float(scale))
    return _BUILD_CACHE[key]


def run_on_hw(in_maps, scale, trace=False):
    from concourse.bass_utils import run_bass_kernel_spmd

    nc = _cached_nc(scale)
    return run_bass_kernel_spmd(nc, in_maps, list(range(NCORES)), trace=trace)


def assemble(results):
    out = np.empty((B, S, D), dtype=np.float32)
    for c in range(NCORES):
        b, r = divmod(c, GPB)
        out[b, r * SPB:(r + 1) * SPB] = results[c]["y"].astype(np.float32)
    return out


def kernel(x, w_q, w_k, w_v, w_out, softmax_scale):
    scale = float(np.asarray(softmax_scale).reshape(-1)[0])
    in_maps = make_in_maps(x, w_q, w_k, w_v, w_out)
    res = run_on_hw(in_maps, scale, trace=False)
    return assemble(res.results)


# revision 36
# speedup vs baseline: 1.1723x; 1.0465x over previous
"""Multi-head attention (multiquery K/V) Bass kernel for 8 trn2 NeuronCores.

Sharding: 8 cores = 2 batches x 4 query-row quarters. Each core computes the
full multiquery K/V projections for its batch (cheap, dk=64) and runs
attention + output projection for its 512 query rows over all 16 heads.
Output is a pure concatenation across cores -- no collectives.

Design (v5):
- Steady state is jointly PE/ACT bound: per t-block the PE runs 2 score +
  2 attn@V matmuls (512 cols, bf16) plus ~1 amortized projection matmul;
  the Scalar engine one exp over [128, 2, 512].
- Scores keep the full-128 contraction with the zero padding in the
  STATIONARY operand (K2Tz[:,0]=[K;0], K2Tz[:,1]=[0;K]); qz holds the head
  pair stacked [Qe;Qo] -- no big memzero, single evacuation per q-block.
- K and V projections fuse into one stationary ([w_k.T | w_v.T], 128 wide):
  one 8-matmul pass per x block yields K.T and V.T together, halving the
  projection matmuls of v3.  The attn@V denominator ones-column is
  memset-built, not DMA'd.
- attn@V lags exp by 2 t-blocks everywhere (ex bufs=4): the previous pass's
  last two attn@V land in tb0/tb1, the accumulator evacuates (split per j)
  at tb1, the new pass's attn@V starts at tb2.  No matmul burst at pass
  boundaries; hooks are spread so no t-block carries more than 2 extra
  matmuls.
- All input DMAs issue on the single gpsimd queue in exact first-use order
  (multiple queues just split HBM bandwidth and starve the critical path).
  Warm-up matmuls on a junk tile un-throttle the PE HAM and a dummy 1-col
  exp preloads the activation table set during the initial DMA wait.
- Tail: deferred attn@V right after the last exp; the reciprocal and the
  normalize multiplies read the accumulator PSUM directly (no evacuation);
  bc evacuation runs on the now-idle Scalar engine; junk matmuls keep the
  PE clock un-throttled through the normalize; per-block y matmuls ->
  single merged add -> DMA writeback pipeline.
"""

import sys

import numpy as np

if "/opt/trn_rl_repo" not in sys.path:
    sys.path.insert(0, "/opt/trn_rl_repo")

B, S, D = 2, 2048, 1024
H, DK = 16, 64
H2 = H // 2  # head pairs
P = 128
NCORES, GPB = 8, 4
SPB = S // GPB  # 512 query rows per core
KC = D // P  # 8 contraction subtiles over d_model
NT = S // P  # 16 key/t blocks
NSB = SPB // P  # 4 s blocks


def build_bass(scale: float, debug: bool = False):
    import concourse.bacc as bacc
    import concourse.mybir as mybir
    import concourse.tile as tile
    from concourse.bass import ts
    from concourse.dve_ops import (
        RECIP_APPROX_FAST_CONSTS,
        RECIPROCAL_APPROX_FAST,
    )

    fp32 = mybir.dt.float32
    mdt = mybir.dt.float32r
    Act = mybir.ActivationFunctionType

    bf16 = mybir.dt.bfloat16
    nc = bacc.Bacc(None, target_bir_lowering=False)
    xT = nc.dram_tensor("xT", [D, S], bf16, kind="ExternalInput")
    cstb = nc.dram_tensor("cstb", [P, 192], bf16, kind="ExternalInput")
    wqT = nc.dram_tensor("wqT", [D, D], bf16, kind="ExternalInput")
    wkv = nc.dram_tensor("wkv", [D, P], bf16, kind="ExternalInput")
    wo2 = nc.dram_tensor("wo2", [P, H2, D], bf16, kind="ExternalInput")
    y = nc.dram_tensor("y", [SPB, D], bf16, kind="ExternalOutput")

    xT3 = xT.rearrange("(po pi) s -> pi po s", pi=P)
    wq3 = wqT.rearrange("(po pi) d -> pi po d", pi=P)
    wkv3 = wkv.rearrange("(po pi) d -> pi po d", pi=P)

    with tile.TileContext(nc) as tc:
        with (
            tc.tile_pool(name="sb", bufs=1) as sb,
            tc.tile_pool(name="ps", bufs=1, space="PSUM") as ps,
        ):
            # ---- persistent SBUF ----
            cstb_sb = sb.tile([P, 192], bf16, name="cstb")
            K2Tz = sb.tile([P, 2, S], bf16, name="K2Tz")
            # Vp stationary: col 0 = denominator ones column, cols 32:96 = V'
            Vp = sb.tile([P, NT, 96], bf16, name="Vp")
            qz = sb.tile([P, KC, SPB], bf16, name="qz")
            rec1 = sb.tile([1, 2, SPB], mdt, name="rec1")
            rec1b = sb.tile([1, 2, SPB], bf16, name="rec1b")
            acc_sb = sb.tile([96, 2, SPB], fp32, name="acc_sb")
            y_sb = sb.tile([P, NSB, D], fp32, name="y_sb")
            wkv_sb = sb.tile([P, KC, P], bf16, name="wkv")
            wq_sb = sb.tile([P, KC, D], bf16, name="wq")
            wo2_sb = sb.tile([P, H2, D], bf16, name="wo2")
            xc0 = sb.tile([P, KC, SPB], bf16, name="xc0")
            junk = sb.tile([P, SPB], bf16, name="junk")
            dead = sb.tile([P, 8], fp32, name="dead")

            ident = cstb_sb[0:DK, 0:DK]
            ones1 = cstb_sb[0:1, DK:DK + 96]  # [1, 96] of ones (bf16)

            def aux(shape, dtype=fp32):
                return ps.tile(shape, dtype, name="aux", tag="aux", bufs=2)

            def junk_mm(n):
                # PE keep-warm: matmuls on the junk tile into a dead psum
                # slot so the HAM clock gate never sees a >3us idle window
                for _ in range(n):
                    wps = aux([P, SPB])
                    nc.tensor.matmul(wps[:], junk[:, 0:P], junk[:],
                                     start=True, stop=True)

            # ---- warm-up: junk matmuls un-throttle the PE HAM while DMAs
            # stream; a 1-col exp preloads the activation table set ----
            # enough junk matmuls that the PE HAM un-throttles (~3.4us of
            # sustained busy) right as the critical input DMAs land, so the
            # whole prepass chain runs at the warm 2.4 GHz clock
            nc.vector.memset(junk[:], 0)
            junk_mm(12)

            # ---- input DMAs: single gpsimd queue, exact first-use order
            # (parallel queues would split HBM bandwidth and starve the
            # critical path) ----
            # two queues (gpsimd + sync), each ~165 GB/s: the xc0 halves run
            # in parallel and the bulk trails each queue's critical pieces
            xcs = {0: xc0}
            xc1 = sb.tile([P, KC, SPB], bf16, name="xc", tag="xc", bufs=2)
            xc2 = sb.tile([P, KC, SPB], bf16, name="xc", tag="xc", bufs=2)
            xc3 = sb.tile([P, KC, SPB], bf16, name="xc", tag="xc", bufs=2)
            xcs[1], xcs[2], xcs[3] = xc1, xc2, xc3
            nc.gpsimd.dma_start(wkv_sb[:], wkv3[:])
            nc.sync.dma_start(wq_sb[:, :, ts(0, P)], wq3[:, :, ts(0, P)])
            nc.gpsimd.dma_start(xc0[:, 0:4, :], xT3[:, 0:4, 0:SPB])
            nc.sync.dma_start(xc0[:, 4:KC, :], xT3[:, 4:KC, 0:SPB])
            nc.sync.dma_start(wq_sb[:, :, ts(1, P)], wq3[:, :, ts(1, P)])
            nc.gpsimd.dma_start(xc1[:], xT3[:, :, ts(1, SPB)])
            nc.sync.dma_start(cstb_sb[:], cstb[:])
            nc.sync.dma_start(xc2[:], xT3[:, :, ts(2, SPB)])
            nc.gpsimd.dma_start(xc3[:], xT3[:, :, ts(3, SPB)])
            for i in range(H2):
                nc.gpsimd.dma_start(wo2_sb[:, i, :], wo2[:, i, :])
            for i in range(2, KC):
                nc.sync.dma_start(wq_sb[:, :, ts(i, P)],
                                  wq3[:, :, ts(i, P)])

            # act-table preload (reads zeroed junk; result never used)
            nc.scalar.activation(dead[:, 0:1], junk[:, 0:1], Act.Exp,
                                 scale=1.0)

            # ---- one-time constant regions (off the critical path) ----
            nc.vector.memset(K2Tz[DK:P, 0, :], 0)
            nc.vector.memset(K2Tz[0:DK, 1, :], 0)
            for tb in range(NT):
                # denominator ones column; col 1 is in the never-read pad
                # region (2-wide so the AP keeps a free dim)
                nc.vector.memset(Vp[:, tb, 0:2], 1.0)
            # fp32 tile (MEMSET rejects float32r) bitcast to fp32r at use
            ones_m = sb.tile([1, 96], fp32, name="ones_m")
            nc.vector.memset(ones_m[:], 1.0)

            # fused K/V projection emitters, split into <=1us pieces that
            # slot into per-tb PE slack; "a" starts the psum accumulation,
            # "b" finishes it and evacuates K.T / V.T
            pstate = {}

            def kv_a(c):
                kvps = aux([P, SPB])
                pstate[("kv", c)] = kvps
                for k in range(4):
                    nc.tensor.matmul(
                        kvps[:], wkv_sb[:, k, :], xcs[c][:, k, :],
                        start=(k == 0), stop=False,
                    )

            def kv_b(c):
                kvps = pstate.pop(("kv", c))
                for k in range(4, KC):
                    nc.tensor.matmul(
                        kvps[:], wkv_sb[:, k, :], xcs[c][:, k, :],
                        start=False, stop=(k == KC - 1),
                    )
                nc.vector.tensor_copy(K2Tz[0:DK, 0, ts(c, 512)],
                                      kvps[0:DK, :])
                nc.vector.tensor_copy(K2Tz[DK:P, 1, ts(c, 512)],
                                      kvps[0:DK, :])
                vsb = sb.tile([DK, SPB], bf16, name="vsb", tag="vsb", bufs=1)
                pstate[("vsb", c)] = vsb
                nc.vector.tensor_copy(vsb[:], kvps[DK:P, :])

            def v_tr(c):
                # PE-transpose V.T -> V' [t, dv] into Vp cols 32:96
                vsb = pstate.pop(("vsb", c))
                for pair in range(2):
                    trs = [aux([P, DK], bf16), aux([P, DK], bf16)]
                    for q in range(2):
                        nc.tensor.transpose(
                            trs[q][:], vsb[:, ts(2 * pair + q, P)], ident
                        )
                    for q in range(2):
                        nc.vector.tensor_copy(
                            Vp[:, 4 * c + 2 * pair + q, 32:96], trs[q][:]
                        )

            def q_a(m):
                qps = aux([P, SPB])
                pstate[("q", m)] = qps
                for k in range(4):
                    nc.tensor.matmul(
                        qps[:], wq_sb[:, k, ts(m, P)], xc0[:, k, :],
                        start=(k == 0), stop=False,
                    )

            def q_b(m):
                qps = pstate.pop(("q", m))
                for k in range(4, KC):
                    nc.tensor.matmul(
                        qps[:], wq_sb[:, k, ts(m, P)], xc0[:, k, :],
                        start=False, stop=(k == KC - 1),
                    )
                nc.vector.tensor_copy(qz[:, m, :], qps[:])

            # ---- pre-pass: K/V for c0 and Q for m0, m1 ----
            kv_a(0)
            kv_b(0)
            q_a(0)
            q_b(0)
            q_a(1)
            q_b(1)
            v_tr(0)

            # ---- attention passes, one head pair each ----
            def emit_av(acc, tb, ex):
                first, last = (tb == 0), (tb == NT - 1)
                for j in range(2):
                    nc.tensor.matmul(
                        acc[:, j, :], Vp[:, tb, :], ex[:, j, :],
                        start=first, stop=last,
                    )

            def norm_a():
                # reciprocal of the denominator rows; the offset-dropping
                # custom op needs base partition 0 (acc_sb row 0), then a
                # tiny cast so the broadcast matmul streams bf16
                c = RECIP_APPROX_FAST_CONSTS
                nc.vector._custom_dve(
                    RECIPROCAL_APPROX_FAST,
                    out=rec1[0:1, :, :], in0=acc_sb[0:1, :, :],
                    s0=c["s0"], s1=c["s1"], imm2=c["imm2"],
                )
                nc.vector.tensor_copy(rec1b[:], rec1[:])

            def norm_b_j(j, oT):
                # broadcast 1/den across partitions on the (idle) GpSimd
                # engine -- keeps the broadcast off the PE entirely -- then
                # DVE multiplies write the pair-stacked normalized oT.
                # 32-row pieces: spans starting at partition 32/96 may cover
                # at most 32 partitions, and both inputs must share a base
                # partition.
                bcs = sb.tile([96, SPB], bf16, name="bcs", tag="bcs2",
                              bufs=2)
                nc.gpsimd.partition_broadcast(bcs[:], rec1b[0:1, j, :])
                for hf in range(2):
                    nc.vector.tensor_mul(
                        oT[j * DK + hf * 32:j * DK + hf * 32 + 32, :],
                        acc_sb[32 + hf * 32:64 + hf * 32, j, :],
                        bcs[32 + hf * 32:64 + hf * 32, :],
                    )

            def emit_y1(prev_hp, oT, sb4, df):
                yps = aux([P, 512])
                nc.tensor.matmul(
                    yps[:], oT[:, ts(sb4, P)],
                    wo2_sb[:, prev_hp, ts(df, 512)],
                    start=True, stop=True,
                )
                if prev_hp == 0:
                    nc.vector.tensor_copy(y_sb[:, sb4, ts(df, 512)], yps[:])
                else:
                    nc.vector.tensor_add(
                        y_sb[:, sb4, ts(df, 512)], yps[:],
                        y_sb[:, sb4, ts(df, 512)],
                    )

            def q_p1(m, k):
                # single-matmul q-projection piece (chunk k of 8)
                if k == 0:
                    pstate[("q", m)] = aux([P, SPB])
                qps = pstate[("q", m)]
                nc.tensor.matmul(
                    qps[:], wq_sb[:, k, ts(m, P)], xc0[:, k, :],
                    start=(k == 0), stop=(k == KC - 1),
                )

            def q_fin(m):
                qps = pstate.pop(("q", m))
                nc.vector.tensor_copy(qz[:, m, :], qps[:])

            # hook schedule: {pass: {tb: [closures]}}.  Deadlines: K2Tz
            # block c by sc(4c); Vp block c by av(4c) (lag 2); qz m by the
            # next pass's sc(0).  v_tr(c) is emitted >=2 tb before av(4c)
            # so its PE transposes precede the dependent attn@V.
            hooks = {
                0: {1: [lambda: kv_a(1)], 2: [lambda: kv_b(1)],
                    4: [lambda: v_tr(1)],
                    5: [lambda: kv_a(2)], 6: [lambda: kv_b(2)],
                    8: [lambda: v_tr(2)],
                    9: [lambda: kv_a(3)], 10: [lambda: kv_b(3)],
                    12: [lambda: v_tr(3)]},
            }
            # single-matmul q pieces at tb2-9: together with single y pieces
            # at tb12-15 this keeps every t-block at <=1 extra matmul, so
            # the PE stays ahead of the exp cadence and the sc-bank WAR
            # round-trip never binds
            for p in range(1, 7):
                for k in range(KC):
                    hooks.setdefault(p, {}).setdefault(2 + k, []).append(
                        lambda m=p + 1, k=k: q_p1(m, k)
                    )
                hooks.setdefault(p, {}).setdefault(9, []).append(
                    lambda m=p + 1: q_fin(m)
                )

            prev = None  # (hp, acc, tail exs {14, 15})
            for hp in range(H2):
                acc = ps.tile([96, 2, SPB], fp32, name="acc", tag="acc",
                              bufs=1)
                exs = {}
                oT_prev = None
                for tb in range(NT):
                    sc = ps.tile([P, 2, SPB], fp32, name=f"sc{tb % 2}",
                                 tag=f"sc{tb % 2}", bufs=1)
                    for j in range(2):
                        nc.tensor.matmul(
                            sc[:, j, :], K2Tz[:, j, ts(tb, P)], qz[:, hp, :],
                            start=True, stop=True,
                        )
                    # attn@V lags exp by 2; the previous pass's last two
                    # attn@V land in tb0/tb1, its accumulator evacuates at
                    # tb1 (split per j) and this pass's attn@V starts at tb2
                    if tb == 0 and prev is not None:
                        emit_av(prev[1], 14, prev[2].pop(14))
                    if tb == 1 and prev is not None:
                        emit_av(prev[1], 15, prev[2].pop(15))
                    if tb >= 2:
                        emit_av(acc, tb - 2, exs.pop(tb - 2))
                    ex = sb.tile([P, 2, SPB], bf16, name="ex", tag="ex",
                                 bufs=4)
                    nc.scalar.activation(ex[:], sc[:], Act.Exp, scale=scale)
                    exs[tb] = ex
                    if tb == 1 and prev is not None:
                        nc.vector.tensor_copy(acc_sb[:, 0, :],
                                              prev[1][:, 0, :])
                        nc.vector.tensor_copy(acc_sb[:, 1, :],
                                              prev[1][:, 1, :])
                    if prev is not None:
                        # norm/y positions chosen so no PE instruction ever
                        # waits on the DVE normalize chain: bc j0 fires at
                        # tb4 (cast done ~tb3.8), y(0,0) at tb10 (oT done
                        # ~tb7.5); tb12-15 stay light for the pass boundary
                        if tb == 2:
                            norm_a()
                        elif tb == 4:
                            oT_prev = sb.tile([P, SPB], bf16, name="oT",
                                              tag="oT", bufs=2)
                            norm_b_j(0, oT_prev)
                        elif tb == 5:
                            norm_b_j(1, oT_prev)
                        elif tb == 10:
                            emit_y1(prev[0], oT_prev, 0, 0)
                            emit_y1(prev[0], oT_prev, 0, 1)
                        elif tb == 11:
                            emit_y1(prev[0], oT_prev, 1, 0)
                            emit_y1(prev[0], oT_prev, 1, 1)
                        elif tb >= 12:
                            emit_y1(prev[0], oT_prev,
                                    (tb - 8) // 2, (tb - 8) % 2)
                    for fn in hooks.get(hp, {}).get(tb, []):
                        fn()
                prev = (hp, acc, exs)

            # ---- tail: last pass's deferred attn@V + normalize + y +
            # writeback.  The reciprocal and normalize multiplies read the
            # accumulator PSUM directly -- no evacuation; junk matmuls keep
            # the PE clock warm through the DVE chain ----
            y_out = sb.tile([P, NSB, D], bf16, name="y_out")
            pacc = prev[1]
            emit_av(pacc, 14, prev[2].pop(14))
            emit_av(pacc, 15, prev[2].pop(15))
            c = RECIP_APPROX_FAST_CONSTS
            oT_last = sb.tile([P, SPB], bf16, name="oTl", tag="oT", bufs=2)
            bcl, bcsl = {}, {}
            # single reciprocal call over both j: the custom DVE op drops AP
            # offsets on HW, so any slice not at the tile base reads/writes
            # the wrong location (a per-j split raced and corrupted j0)
            nc.vector._custom_dve(
                RECIPROCAL_APPROX_FAST,
                out=rec1[0:1, :, :], in0=pacc[0:1, :, :],
                s0=c["s0"], s1=c["s1"], imm2=c["imm2"],
            )
            for j in range(2):
                # fp32r moving direct from rec1 skips a cast
                bcl[j] = aux([96, SPB])
                nc.tensor.matmul(
                    bcl[j][:], ones_m[:].bitcast(mdt), rec1[0:1, j, :],
                    start=True, stop=True
                )
                bcsl[j] = sb.tile([96, SPB], fp32, name="bcsl", tag="bcsl",
                                  bufs=2)
                nc.scalar.copy(bcsl[j][:], bcl[j][:])
            junk_mm(8)
            for j in range(2):
                for hf in range(2):
                    lo = 32 + hf * 32
                    nc.vector.tensor_mul(
                        oT_last[j * DK + hf * 32:j * DK + hf * 32 + 32, :],
                        pacc[lo:lo + 32, j, :],
                        bcsl[j][lo:lo + 32, :],
                    )
            for sb4 in range(NSB):
                yps = ps.tile([P, D], fp32, name="typs", tag=f"sc{sb4 % 2}",
                              bufs=1)
                for df in range(2):
                    nc.tensor.matmul(
                        yps[:, ts(df, 512)], oT_last[:, ts(sb4, P)],
                        wo2_sb[:, H2 - 1, ts(df, 512)],
                        start=True, stop=True,
                    )
                nc.vector.tensor_add(
                    y_out[:, sb4, :], yps[:], y_sb[:, sb4, :]
                )
                # issue from the Scalar engine: idle after the last exp, and
                # far cheaper per issue than gpsimd
                nc.scalar.dma_start(y[ts(sb4, P), :], y_out[:, sb4, :])

    nc.compile()
    return nc


def make_in_maps(x, w_q, w_k, w_v, w_out):
    import ml_dtypes

    bf16 = ml_dtypes.bfloat16
    cstb = np.zeros((P, 192), dtype=np.float32)
    cstb[0:DK, 0:DK] = np.eye(DK, dtype=np.float32)
    cstb[:, DK:192] = 1.0
    cstb = cstb.astype(bf16)
    x = np.ascontiguousarray(np.asarray(x, dtype=np.float32))
    w_q = np.asarray(w_q, dtype=np.float32)
    w_k = np.asarray(w_k, dtype=np.float32)
    w_v = np.asarray(w_v, dtype=np.float32)
    w_out = np.asarray(w_out, dtype=np.float32)

    wqT = np.ascontiguousarray(w_q.T.astype(bf16))
    wkv = np.ascontiguousarray(
        np.concatenate([w_k.T, w_v.T], axis=1).astype(bf16)
    )
    # head-pair-stacked w_out.T: wo2[phi*64+dv, hp, d] = w_out.T[(2hp+phi)*64+dv, d]
    wo2 = np.ascontiguousarray(
        w_out.T.reshape(H2, 2, DK, D).transpose(1, 2, 0, 3).reshape(P, H2, D)
        .astype(bf16)
    )

    in_maps = []
    for c in range(NCORES):
        b, r = divmod(c, GPB)
        # roll this core's query rows to the front; t-order is irrelevant
        # (attention sums over t), so K/V are unaffected
        xb = np.roll(x[b], -r * SPB, axis=0)
        xTc = np.ascontiguousarray(xb.T.astype(bf16))
        in_maps.append(
            {"xT": xTc, "wqT": wqT, "wkv": wkv, "wo2": wo2, "cstb": cstb}
        )
    return in_maps


_BUILD_CACHE = {}


def _cached_nc(scale: float):
    key = round(float(scale), 12)
    if key not in _BUILD_CACHE:
        _BUILD_CACHE[key] = build_bass(float(scale))
    return _BUILD_CACHE[key]


def run_on_hw(in_maps, scale, trace=False):
    from concourse.bass_utils import run_bass_kernel_spmd

    nc = _cached_nc(scale)
    return run_bass_kernel_spmd(nc, in_maps, list(range(NCORES)), trace=trace)


def assemble(results):
    out = np.empty((B, S, D), dtype=np.float32)
    for c in range(NCORES):
        b, r = divmod(c, GPB)
        out[b, r * SPB:(r + 1) * SPB] = results[c]["y"].astype(np.float32)
    return out


def kernel(x, w_q, w_k, w_v, w_out, softmax_scale):
    scale = float(np.asarray(softmax_scale).reshape(-1)[0])
    in_maps = make_in_maps(x, w_q, w_k, w_v, w_out)
    res = run_on_hw(in_maps, scale, trace=False)
    return assemble(res.results)
